# revision 1
# baseline (speedup 1.0000x reference)
"""Trainium2 Bass kernel for pooled cross-attention block (dense_transformer).

Reference computation per batch element b (B=8, one per NeuronCore):
  x2p = 2x2 mean-pool(x2)                      [512, 32, 32]
  Q = Wq @ x1  + bq                            [64, 4096]   (d-part layout)
  K = Wk @ x2p + bk                            [64, 1024]
  V = Wv @ x2p + bv                            [64, 1024]
  attn = softmax_n(Q^T K)                      [4096, 1024]
  out  = attn @ V^T                            [4096, 64]
  y    = out @ Wo^T + bo -> [256, 4096] ; result = x1 + y

Kernel strategy (all on-chip per core, streamed over n in 512-col chunks):
  - scores computed TRANSPOSED: sT[m, n] = K^T Q so softmax's reduce dim m
    is the partition dim; the row-sum r[n] is obtained for free by
    augmenting V^T with a ones column (row 64 of the U = V_aug^T expS
    accumulation).  No PE transposes anywhere.
  - bias algebra (all exact): bk drops (per-row softmax shift invariance);
    bq folded into Q via per-partition bias on the PSUM->SBUF copy;
    bv folded into bo' = bo + Wo@bv on host (attn rows sum to 1);
    bo' enters via the ones-row of the normalized U (row 64 == 1 after
    dividing by r) against an augmented Wo^T.
  - 2x2 pooling: two strided DVE adds; the 1/4 scale is folded into Wk/Wv
    on the host.
  - dtypes: Q projection runs float32r on raw fp32 x1 (full PE rate at
    N=512, no cast cost); everything downstream of the projections is
    bf16 on the PE with fp32 PSUM accumulation.
"""

import sys

for _p in ("/opt/trn_rl_repo",):
    if _p not in sys.path:
        sys.path.insert(0, _p)

import numpy as np

B, C1, C2, H, W, D = 8, 256, 512, 64, 64, 64
HW = H * W            # n (query) size: 4096
M = (H // 2) * (W // 2)  # kv size: 1024
NCH = 512             # n-chunk (one fp32 PSUM bank)
NCHUNKS = HW // NCH   # 8
C1T = C1 // 128       # 2
C2T = C2 // 128       # 4
MT = M // 128         # 8

_CACHE = {}


def _build():
    import concourse.bass as bass
    import concourse.tile as tile
    from concourse import bacc, mybir

    dt = mybir.dt
    f32, bf16, f32r = dt.float32, dt.bfloat16, dt.float32r
    Exp = mybir.ActivationFunctionType.Exp

    nc = bacc.Bacc(
        "TRN2", target_bir_lowering=False, debug=False, num_devices=8
    )
    x1 = nc.dram_tensor("x1", [C1, HW], f32r, kind="ExternalInput").ap()
    x2 = nc.dram_tensor("x2", [C2, HW], f32, kind="ExternalInput").ap()
    wqt = nc.dram_tensor("wqt", [C1T, 128, D], f32r, kind="ExternalInput").ap()
    wkt = nc.dram_tensor("wkt", [C2T, 128, D], bf16, kind="ExternalInput").ap()
    wvt = nc.dram_tensor("wvt", [C2T, 128, D], bf16, kind="ExternalInput").ap()
    wot = nc.dram_tensor("wot", [D + 1, C1], bf16, kind="ExternalInput").ap()
    bq = nc.dram_tensor("bq", [D, 1], f32, kind="ExternalInput").ap()
    out = nc.dram_tensor("out", [C1, HW], f32, kind="ExternalOutput").ap()

    from contextlib import ExitStack

    with tile.TileContext(nc) as tc, ExitStack() as ctx:
        pool = lambda name, bufs, **kw: ctx.enter_context(
            tc.tile_pool(name=name, bufs=bufs, **kw)
        )
        consts = pool("consts", 1)
        x2p = pool("x2p", 8)
        poolp = pool("poolp", 8)
        sp = pool("sp", 8)
        kvsb = pool("kvsb", 1)
        x1p = pool("x1p", 8)
        qsbp = pool("qsbp", 2)
        esp = pool("esp", 3)
        rp = pool("rp", 2)
        onp = pool("onp", 2)
        youtp = pool("youtp", 6)
        ps_a = pool("ps_a", 2, space="PSUM")
        ps_b = pool("ps_b", 2, space="PSUM")
        ps_s = pool("ps_s", 2, space="PSUM")
        ps_y = pool("ps_y", 2, space="PSUM")

        # ---- constants -------------------------------------------------
        wqt_sb = consts.tile([128, C1T, D], f32r, tag="wqt")
        for t in range(C1T):
            nc.sync.dma_start(out=wqt_sb[:, t, :], in_=wqt[t])
        wkt_sb = consts.tile([128, C2T, D], bf16, tag="wkt")
        wvt_sb = consts.tile([128, C2T, D], bf16, tag="wvt")
        for t in range(C2T):
            nc.sync.dma_start(out=wkt_sb[:, t, :], in_=wkt[t])
            nc.sync.dma_start(out=wvt_sb[:, t, :], in_=wvt[t])
        wot_sb = consts.tile([D + 1, C1], bf16, tag="wot")
        nc.sync.dma_start(out=wot_sb, in_=wot)
        bq_sb = consts.tile([D, 1], f32, tag="bq")
        nc.sync.dma_start(out=bq_sb, in_=bq)

        # ---- phase A: pool x2, project K and V^T -----------------------
        k_ps = [ps_a.tile([D, NCH], f32, tag="kq", name=f"k_ps{h}") for h in range(2)]
        v_ps = ps_b.tile([128, MT, D], f32, tag="uv", name="v_ps")
        half = HW // 2
        for ci in range(C2T):
            for hi in range(2):
                x2t = x2p.tile([128, half], f32, tag="x2t", name="x2t")
                nc.sync.dma_start(
                    out=x2t,
                    in_=x2[ci * 128:(ci + 1) * 128, hi * half:(hi + 1) * half],
                )
                x2v = x2t.rearrange("p (h w2 two) -> p h w2 two", w2=W // 2, two=2)
                t1 = poolp.tile([128, H // 2, W // 2], f32, tag="t1", name="t1")
                nc.vector.tensor_add(t1, x2v[:, :, :, 0], x2v[:, :, :, 1])
                t1v = t1.rearrange("p (h2 two) w2 -> p h2 two w2", two=2)
                s_bf = sp.tile([128, NCH], bf16, tag="s", name="s_bf")
                s3 = s_bf.rearrange("p (h2 w2) -> p h2 w2", h2=H // 4)
                nc.vector.tensor_add(s3, t1v[:, :, 0, :], t1v[:, :, 1, :])
                nc.tensor.matmul(
                    k_ps[hi],
                    lhsT=wkt_sb[:, ci, :],
                    rhs=s_bf,
                    start=(ci == 0),
                    stop=(ci == C2T - 1),
                )
                for mj in range(MT // 2):
                    mi = hi * (MT // 2) + mj
                    nc.tensor.matmul(
                        v_ps[:, mi, :],
                        lhsT=s_bf[:, mj * 128:(mj + 1) * 128],
                        rhs=wvt_sb[:, ci, :],
                        start=(ci == 0),
                        stop=(ci == C2T - 1),
                    )
        k_sb = kvsb.tile([D, M], bf16, tag="ksb")
        for h in range(2):
            nc.vector.tensor_copy(k_sb[:, h * NCH:(h + 1) * NCH], k_ps[h])
        v_aug = kvsb.tile([128, MT, D + 1], bf16, tag="vaug")
        nc.vector.memset(v_aug[:, :, D], 1.0)
        for mi in range(MT):
            nc.vector.tensor_copy(v_aug[:, mi, 0:D], v_ps[:, mi, :])

        # ---- phase B: stream n-chunks ----------------------------------
        for nj in range(NCHUNKS):
            nsl = slice(nj * NCH, (nj + 1) * NCH)
            x1t = x1p.tile([128, C1T, NCH], f32r, tag="x1t", name="x1t")
            for t in range(C1T):
                nc.sync.dma_start(out=x1t[:, t, :], in_=x1[t * 128:(t + 1) * 128, nsl])
            q_ps = ps_a.tile([D, NCH], f32, tag="kq", name="q_ps")
            for t in range(C1T):
                nc.tensor.matmul(
                    q_ps,
                    lhsT=wqt_sb[:, t, :],
                    rhs=x1t[:, t, :],
                    start=(t == 0),
                    stop=(t == C1T - 1),
                )
            q_sb = qsbp.tile([D, NCH], bf16, tag="qsb", name="q_sb")
            nc.scalar.add(q_sb, q_ps, bq_sb)

            u_ps = ps_b.tile([D + 1, NCH], f32, tag="uv", name="u_ps")
            for mi in range(MT):
                s_ps = ps_s.tile([128, NCH], f32, tag="st", name="s_ps")
                nc.tensor.matmul(
                    s_ps,
                    lhsT=k_sb[:, mi * 128:(mi + 1) * 128],
                    rhs=q_sb,
                    start=True,
                    stop=True,
                )
                es = esp.tile([128, NCH], bf16, tag="es", name="es")
                nc.scalar.activation(es, s_ps, Exp)
                nc.tensor.matmul(
                    u_ps,
                    lhsT=v_aug[:, mi, :],
                    rhs=es,
                    start=(mi == 0),
                    stop=(mi == MT - 1),
                )
            rinv = rp.tile([1, NCH], f32, tag="rinv", name="rinv")
            nc.vector.reciprocal(rinv, u_ps[D:D + 1, :])
            rb = rp.tile([D + 1, NCH], f32, tag="rb", name="rb")
            nc.gpsimd.partition_broadcast(rb, rinv)
            on = onp.tile([D + 1, NCH], bf16, tag="on", name="on")
            nc.vector.tensor_mul(on, u_ps, rb)
            for t in range(C1T):
                y_ps = ps_y.tile([128, NCH], f32, tag="y", name="y_ps")
                nc.tensor.matmul(
                    y_ps,
                    lhsT=wot_sb[:, t * 128:(t + 1) * 128],
                    rhs=on,
                    start=True,
                    stop=True,
                )
                yo = youtp.tile([128, NCH], f32, tag="yo", name="yo")
                nc.vector.tensor_add(yo, x1t[:, t, :].bitcast(f32), y_ps)
                nc.sync.dma_start(out=out[t * 128:(t + 1) * 128, nsl], in_=yo)
    nc.compile()
    return nc


def _get_nc():
    if "nc" not in _CACHE:
        _CACHE["nc"] = _build()
    return _CACHE["nc"]


def _prep_in_maps(x1, x2, Wq, bq, Wk, bk, Wv, bv, Wo, bo):
    import ml_dtypes

    bf16 = ml_dtypes.bfloat16
    f32 = np.float32
    x1 = np.asarray(x1, f32)
    x2 = np.asarray(x2, f32)
    Wq = np.asarray(Wq, f32)
    Wk = np.asarray(Wk, f32)
    Wv = np.asarray(Wv, f32)
    Wo = np.asarray(Wo, f32)
    bq = np.asarray(bq, f32)
    bk = np.asarray(bk, f32)
    bv = np.asarray(bv, f32)
    bo = np.asarray(bo, f32)

    wqt = np.ascontiguousarray(Wq.T.reshape(C1T, 128, D))
    wkt = np.ascontiguousarray((0.25 * Wk).T.reshape(C2T, 128, D)).astype(bf16)
    wvt = np.ascontiguousarray((0.25 * Wv).T.reshape(C2T, 128, D)).astype(bf16)
    # bk is softmax-invariant (constant per score row) and is dropped.
    # bv folds into the output bias because attention rows sum to one.
    bo_eff = bo + Wo @ bv
    wot = np.ascontiguousarray(
        np.concatenate([Wo.T, bo_eff[None, :]], axis=0)
    ).astype(bf16)
    bqv = np.ascontiguousarray(bq.reshape(D, 1))

    shared = {"wqt": wqt, "wkt": wkt, "wvt": wvt, "wot": wot, "bq": bqv}
    in_maps = []
    for b in range(B):
        m = dict(shared)
        m["x1"] = np.ascontiguousarray(x1[b].reshape(C1, HW))
        m["x2"] = np.ascontiguousarray(x2[b].reshape(C2, HW))
        in_maps.append(m)
    return in_maps


def run(inputs, trace=False, **trace_kwargs):
    from concourse.bass_utils import run_bass_kernel_spmd

    nc = _get_nc()
    in_maps = _prep_in_maps(**inputs)
    res = run_bass_kernel_spmd(
        nc, in_maps, list(range(B)), trace=trace, **trace_kwargs
    )
    out = np.stack([res.results[i]["out"] for i in range(B)])
    out = out.reshape(B, C1, H, W).astype(np.float32)
    return out, res


def kernel(**inputs) -> np.ndarray:
    out, _ = run(inputs, trace=False)
    return out



# revision 23
# speedup vs baseline: 1.1721x; 1.1721x over previous
"""Trainium2 Bass kernel for pooled cross-attention block (dense_transformer).

Reference computation per batch element b (B=8, one per NeuronCore):
  x2p = 2x2 mean-pool(x2)                      [512, 32, 32]
  Q = Wq @ x1  + bq                            [64, 4096]   (d-part layout)
  K = Wk @ x2p + bk                            [64, 1024]
  V = Wv @ x2p + bv                            [64, 1024]
  attn = softmax_n(Q^T K)                      [4096, 1024]
  out  = attn @ V^T                            [4096, 64]
  y    = out @ Wo^T + bo -> [256, 4096] ; result = x1 + y

Kernel strategy (all on-chip per core, streamed over n in 512-col chunks):
  - scores computed TRANSPOSED: sT[m, n] = K^T Q so softmax's reduce dim m
    is the partition dim; the row-sum r[n] is obtained for free by
    augmenting V^T with a ones column (row 64 of the U = V_aug^T expS
    accumulation).  No PE transposes anywhere.
  - bias algebra (all exact): bk drops (per-row softmax shift invariance);
    bq folded into Q on the PSUM->SBUF copy (DVE tensor_scalar);
    bv folded into bo' = bo + Wo@bv on host (attn rows sum to 1);
    bo' enters via the ones-row of the normalized U against an augmented
    Wo^T.
  - K and Q are produced twice (column-group-tiled matmuls run concurrently
    in the PE array, so the duplicate is ~free) so the scores matmuls can be
    issued as row-group-tiled CONCURRENT pairs: mi even uses array rows
    0-63 / K copy 1 / Q copy 1, mi odd uses rows 64-127 / the duplicates.
    Each pair lands in one 2-bank-wide PSUM tile, consumed by a single
    1024-wide exp ACTIVATE.
  - PE clock: the HAM activity monitor keeps the PE at 1.2 GHz unless it
    sees sustained back-to-back matmul activity.  A dense warmup burst at
    t=0 (during the input DMAs) plus a trickle of dummy matmuls between
    the DMA-paced phase-A bursts keeps the array at 2.4 GHz.
  - 2x2 pooling: two strided DVE adds (bf16); the 1/4 scale is folded into
    Wk/Wv on the host.
  - softmax normalization: 1/r via the fast custom-DVE reciprocal
    (~18 bits), broadcast on GpSimd, applied on DVE; the output projection
    matmuls for chunk j are interleaved into chunk j+1's PE stream so the
    PE never waits on the normalization tail.
"""

import sys

for _p in ("/opt/trn_rl_repo",):
    if _p not in sys.path:
        sys.path.insert(0, _p)

import numpy as np

B, C1, C2, H, W, D = 8, 256, 512, 64, 64, 64
HW = H * W            # n (query) size: 4096
M = (H // 2) * (W // 2)  # kv size: 1024
NCH = 512             # n-chunk (one fp32 PSUM bank)
NCHUNKS = HW // NCH   # 8
C1T = C1 // 128       # 2
C2T = C2 // 128       # 4
MT = M // 128         # 8

WARMUP_MMS = 16       # dense burst at t=0 (crosses the ~3.4us HAM window)
TRICKLE_MMS = 8       # dummy MMs after each phase-A group to keep HAM warm

DBG = False           # add intermediate-dump outputs (debugging only)

_CACHE = {}


def _build():
    import concourse.bass as bass
    import concourse.tile as tile
    from concourse import bacc, mybir

    dt = mybir.dt
    f32, bf16, f32r = dt.float32, dt.bfloat16, dt.float32r
    Exp = mybir.ActivationFunctionType.Exp

    nc = bacc.Bacc(
        "TRN2", target_bir_lowering=False, debug=False, num_devices=8
    )
    x1 = nc.dram_tensor("x1", [C1, HW], f32r, kind="ExternalInput").ap()
    x2 = nc.dram_tensor("x2", [C2, HW], f32, kind="ExternalInput").ap()
    # packed weights: one bf16 blob + one f32 blob -> 2 DMAs total
    wkvo = nc.dram_tensor("wkvo", [128, 896], bf16, kind="ExternalInput").ap()
    bqd = nc.dram_tensor("bqd", [128, 1], f32, kind="ExternalInput").ap()
    out = nc.dram_tensor("out", [C1, HW], f32, kind="ExternalOutput").ap()
    if DBG:
        d_ksb = nc.dram_tensor("d_ksb", [128, M], bf16, kind="ExternalOutput").ap()
        d_qall = nc.dram_tensor(
            "d_qall", [128, NCHUNKS, NCH], bf16, kind="ExternalOutput"
        ).ap()
        d_es = nc.dram_tensor(
            "d_es", [128, 4, 2 * NCH], bf16, kind="ExternalOutput"
        ).ap()
        d_on = nc.dram_tensor("d_on", [D + 1, NCH], bf16, kind="ExternalOutput").ap()
        d_rinv = nc.dram_tensor("d_rinv", [1, NCH], f32, kind="ExternalOutput").ap()
        d_r = nc.dram_tensor("d_r", [1, NCH], f32, kind="ExternalOutput").ap()
        d_x1b = nc.dram_tensor(
            "d_x1b", [128, C1T, NCH], bf16, kind="ExternalOutput"
        ).ap()

    x1v = x1.rearrange("(t p) n -> p t n", p=128)    # [128, 2, HW]
    outv = out.rearrange("(t p) n -> p t n", p=128)  # [128, 2, HW]

    from contextlib import ExitStack

    with tile.TileContext(nc) as tc, ExitStack() as ctx:
        pool = lambda name, bufs, **kw: ctx.enter_context(
            tc.tile_pool(name=name, bufs=bufs, **kw)
        )
        consts = pool("consts", 1)
        warm = pool("warm", 1)
        x2st = pool("x2st", 4)
        x1p = pool("x1p", 8)
        x1bp = pool("x1bp", 4)
        poolp = pool("poolp", 2)
        sbfp = pool("sbfp", 2)
        kvsb = pool("kvsb", 1)
        qallp = pool("qallp", 1)
        esp = pool("esp", 3)
        rp = pool("rp", 2)
        rbp = pool("rbp", 2)
        onp = pool("onp", 2)
        youtp = pool("youtp", 4)
        ps_s = pool("ps_s", 2, space="PSUM")   # [128,1024] wide: 2 banks x 2
        ps_u = pool("ps_u", 2, space="PSUM")   # 1 bank x 2
        ps_q = pool("ps_q", 1, space="PSUM")   # 1 bank
        ps_y = pool("ps_y", 1, space="PSUM")   # 1 bank

        # ---- t=0: preload exp table, warmup tiles -----------------------
        actw = warm.tile([1, 8], f32, tag="actw")
        nc.vector.memset(actw, 0.0)
        actw2 = warm.tile([1, 8], f32, tag="actw2")
        nc.scalar.activation(actw2, actw, Exp)

        wu_w = warm.tile([128, 64], bf16, tag="wu_w")
        nc.vector.memset(wu_w, 0.0)
        wu_x = warm.tile([128, NCH], bf16, tag="wu_x")
        nc.vector.memset(wu_x, 0.0)

        # ---- constants (2 DMAs) ----------------------------------------
        wkvo_sb = consts.tile([128, 896], bf16, tag="wkvo")
        nc.sync.dma_start(out=wkvo_sb, in_=wkvo)
        bq_sb = consts.tile([128, 1], f32, tag="bqd")
        nc.sync.dma_start(out=bq_sb, in_=bqd)
        wkt_sb = wkvo_sb[:, 0:256].rearrange("p (c d) -> p c d", c=C2T)
        wvt_sb = wkvo_sb[:, 256:512].rearrange("p (c d) -> p c d", c=C2T)
        wot_sb = wkvo_sb[:, 512:768]              # [65 used, 256]
        wqt_sb = wkvo_sb[:, 768:896].rearrange("p (t d) -> p t d", t=C1T)

        # ---- input DMAs: x2 first (4 x 2MiB), then all of x1 (8) -------
        x2t = [None] * C2T
        for ci in range(C2T):
            x2t[ci] = x2st.tile([128, HW], f32, tag="x2t", name=f"x2t{ci}")
            nc.sync.dma_start(
                out=x2t[ci], in_=x2[ci * 128:(ci + 1) * 128, :]
            )
        x1t = [None] * NCHUNKS
        for nj in range(NCHUNKS):
            x1t[nj] = x1p.tile(
                [128, C1T, NCH], f32r, tag="x1t", name=f"x1t{nj}"
            )
            nc.sync.dma_start(
                out=x1t[nj], in_=x1v[:, :, nj * NCH:(nj + 1) * NCH]
            )

        # ---- PE warmup burst (keeps HAM at 2.4 GHz) --------------------
        wu_ps = ps_y.tile([128, NCH], f32, tag="y", name="wu_ps")
        for i in range(WARMUP_MMS):
            nc.tensor.matmul(
                wu_ps[0:64, :], lhsT=wu_w, rhs=wu_x, start=True, stop=True
            )

        # ---- phase A: pool x2, project K (x2 col-dup) and V^T ----------
        k_pack = ps_s.tile([128, 2 * NCH], f32, tag="s", name="k_pack")
        v_ps = ps_u.tile([128, MT, D], f32, tag="u", name="v_ps")
        for ci in range(C2T):
            x2v = x2t[ci].rearrange(
                "p (h w2 two) -> p h w2 two", w2=W // 2, two=2
            )
            t1 = poolp.tile([128, H, W // 2], bf16, tag="t1", name="t1")
            nc.vector.tensor_add(t1, x2v[:, :, :, 0], x2v[:, :, :, 1])
            t1v = t1.rearrange("p (h2 two) w2 -> p h2 two w2", two=2)
            s_bf = sbfp.tile([128, M], bf16, tag="s", name="s_bf")
            s3 = s_bf.rearrange("p (h2 w2) -> p h2 w2", h2=H // 2)
            nc.vector.tensor_add(s3, t1v[:, :, 0, :], t1v[:, :, 1, :])
            first, last = ci == 0, ci == C2T - 1
            for h in range(2):
                rhs = s_bf[:, h * NCH:(h + 1) * NCH]
                # col-group pair: rows 0-63 and 64-127 of k_pack run
                # concurrently in the PE array
                nc.tensor.matmul(
                    k_pack[0:64, h * NCH:(h + 1) * NCH],
                    lhsT=wkt_sb[:, ci, :], rhs=rhs, start=first, stop=last,
                    skip_group_check=True,
                )
                nc.tensor.matmul(
                    k_pack[64:128, h * NCH:(h + 1) * NCH],
                    lhsT=wkt_sb[:, ci, :], rhs=rhs, start=first, stop=last,
                    skip_group_check=True,
                )
            for mj in range(MT):
                nc.tensor.matmul(
                    v_ps[:, mj, :],
                    lhsT=s_bf[:, mj * 128:(mj + 1) * 128],
                    rhs=wvt_sb[:, ci, :],
                    start=first, stop=last,
                    skip_group_check=True,
                )
            # trickle: keep the PE busy across the DMA-paced gap
            for i in range(TRICKLE_MMS):
                nc.tensor.matmul(
                    wu_ps[0:64, :], lhsT=wu_w, rhs=s_bf[:, 0:NCH],
                    start=True, stop=True,
                )
        k_sb = kvsb.tile([128, M], bf16, tag="ksb")
        # per-bank reads: a single PSUM AP must not cross a 2KB bank
        nc.vector.tensor_copy(k_sb[:, 0:NCH], k_pack[:, 0:NCH])
        nc.vector.tensor_copy(k_sb[:, NCH:2 * NCH], k_pack[:, NCH:2 * NCH])
        v_aug = kvsb.tile([128, MT, D + 1], bf16, tag="vaug")
        nc.vector.memset(v_aug[:, :, D], 1.0)
        nc.vector.tensor_copy(v_aug[:, :, 0:D], v_ps)

        # ---- Q projection (col-dup pair, bf16 via GpSimd cast) ---------
        q_all = qallp.tile([128, NCHUNKS, NCH], bf16, tag="qall")
        x1b = [None] * NCHUNKS

        def x1_cast(nj):
            x1b[nj] = x1bp.tile(
                [128, C1T, NCH], bf16, tag="x1b", name=f"x1b{nj}"
            )
            nc.gpsimd.tensor_copy(x1b[nj], x1t[nj].bitcast(f32))

        def q_proj(nj):
            q_ps = ps_q.tile([128, NCH], f32, tag="q", name=f"q_ps{nj}")
            for t in range(C1T):
                nc.tensor.matmul(
                    q_ps[0:64, :], lhsT=wqt_sb[:, t, :], rhs=x1b[nj][:, t, :],
                    start=(t == 0), stop=(t == C1T - 1),
                    skip_group_check=True,
                )
                nc.tensor.matmul(
                    q_ps[64:128, :], lhsT=wqt_sb[:, t, :], rhs=x1b[nj][:, t, :],
                    start=(t == 0), stop=(t == C1T - 1),
                    skip_group_check=True,
                )
            nc.vector.tensor_scalar_add(q_all[:, nj, :], q_ps, bq_sb)

        for nj in range(3):
            x1_cast(nj)
        q_proj(0)
        q_proj(1)

        # ---- phase B: stream n-chunks ----------------------------------
        # PE stream per chunk (software-pipelined so the PE never waits on
        # exp): sp0 sp1 up0 sp2 up1 sp3 [y0'] up2 [y1'] up3, where y' are
        # the previous chunk's output projections.
        pend = None  # (u_ps, nj) awaiting tail + output projection

        def tail_begin(p):
            # normalization tail of the previous chunk (DVE/GpSimd queues)
            u_prev, njp = p
            r_sb = rp.tile([1, NCH], f32, tag="rsb", name="r_sb")
            nc.vector.tensor_copy(r_sb, u_prev[D:D + 1, :])
            rinv = rp.tile([1, NCH], f32, tag="rinv", name="rinv")
            nc.vector.reciprocal_approx_fast(rinv, r_sb)
            if DBG and njp == 0:
                nc.sync.dma_start(out=d_r, in_=r_sb)
                nc.sync.dma_start(out=d_rinv, in_=rinv)
            rb = rbp.tile([D + 1, NCH], f32, tag="rb", name="rb")
            nc.gpsimd.partition_broadcast(rb, rinv)
            on = onp.tile([D + 1, NCH], bf16, tag="on", name="on")
            nc.vector.tensor_mul(on, u_prev, rb)
            if DBG and njp == 0:
                nc.sync.dma_start(out=d_on, in_=on)
            yo = youtp.tile([128, C1T, NCH], f32, tag="yo", name="yo")
            return on, yo, njp

        def tail_y(t, on, yo, njp):
            y_ps = ps_y.tile([128, NCH], f32, tag="y", name=f"y_ps{njp}_{t}")
            nc.tensor.matmul(
                y_ps, lhsT=wot_sb[0:D + 1, t * 128:(t + 1) * 128], rhs=on,
                start=True, stop=True,
            )
            nc.vector.tensor_add(yo[:, t, :], x1t[njp][:, t, :].bitcast(f32), y_ps)

        def tail_flush(on, yo, njp):
            nc.sync.dma_start(
                out=outv[:, :, njp * NCH:(njp + 1) * NCH], in_=yo
            )

        for nj in range(NCHUNKS):
            if nj + 3 < NCHUNKS:
                x1_cast(nj + 3)
            if nj + 2 < NCHUNKS:
                q_proj(nj + 2)
            u_ps = ps_u.tile([D + 1, NCH], f32, tag="u", name=f"u_ps{nj}")
            tl = tail_begin(pend) if pend is not None else None

            s_w = [None] * 4
            es = [None] * 4

            def s_pair(p):
                s_w[p] = ps_s.tile([128, 2 * NCH], f32, tag="s", name=f"s_w{nj}_{p}")
                mi_a, mi_b = 2 * p, 2 * p + 1
                nc.tensor.matmul(
                    s_w[p][:, 0:NCH],
                    lhsT=k_sb[0:64, mi_a * 128:(mi_a + 1) * 128],
                    rhs=q_all[0:64, nj, :], start=True, stop=True,
                )
                nc.tensor.matmul(
                    s_w[p][:, NCH:2 * NCH],
                    lhsT=k_sb[64:128, mi_b * 128:(mi_b + 1) * 128],
                    rhs=q_all[64:128, nj, :], start=True, stop=True,
                )
                es[p] = esp.tile([128, 2 * NCH], bf16, tag="es", name=f"es{nj}_{p}")
                nc.scalar.activation(es[p][:, 0:NCH], s_w[p][:, 0:NCH], Exp)
                nc.scalar.activation(es[p][:, NCH:2 * NCH], s_w[p][:, NCH:2 * NCH], Exp)
                if DBG and nj == 0:
                    nc.sync.dma_start(out=d_es[:, p, :], in_=es[p])

            def u_pair(p):
                mi_a, mi_b = 2 * p, 2 * p + 1
                nc.tensor.matmul(
                    u_ps, lhsT=v_aug[:, mi_a, :], rhs=es[p][:, 0:NCH],
                    start=(p == 0), stop=False,
                )
                nc.tensor.matmul(
                    u_ps, lhsT=v_aug[:, mi_b, :], rhs=es[p][:, NCH:2 * NCH],
                    start=False, stop=(p == 3),
                )

            s_pair(0)
            s_pair(1)
            u_pair(0)
            s_pair(2)
            u_pair(1)
            s_pair(3)
            if tl is not None:
                tail_y(0, *tl)
            u_pair(2)
            if tl is not None:
                tail_y(1, *tl)
                tail_flush(*tl)
            u_pair(3)
            pend = (u_ps, nj)

        tl = tail_begin(pend)
        tail_y(0, *tl)
        tail_y(1, *tl)
        tail_flush(*tl)
        if DBG:
            nc.sync.dma_start(out=d_ksb, in_=k_sb)
            nc.sync.dma_start(out=d_qall, in_=q_all)
            nc.sync.dma_start(out=d_x1b, in_=x1b[0])
    nc.compile()
    return nc


def _get_nc():
    if "nc" not in _CACHE:
        _CACHE["nc"] = _build()
    return _CACHE["nc"]


def _prep_in_maps(x1, x2, Wq, bq, Wk, bk, Wv, bv, Wo, bo):
    import ml_dtypes

    bf16 = ml_dtypes.bfloat16
    f32 = np.float32
    x1 = np.asarray(x1, f32)
    x2 = np.asarray(x2, f32)
    Wq = np.asarray(Wq, f32)
    Wk = np.asarray(Wk, f32)
    Wv = np.asarray(Wv, f32)
    Wo = np.asarray(Wo, f32)
    bq = np.asarray(bq, f32)
    bk = np.asarray(bk, f32)
    bv = np.asarray(bv, f32)
    bo = np.asarray(bo, f32)

    # bk is softmax-invariant (constant per score row) and is dropped.
    # bv folds into the output bias because attention rows sum to one.
    bo_eff = bo + Wo @ bv

    def to_p_inner(w):  # [CT*128, D] -> [128, CT*D]
        ct = w.shape[0] // 128
        return np.ascontiguousarray(
            w.reshape(ct, 128, D).transpose(1, 0, 2).reshape(128, ct * D)
        )

    wkt_p = to_p_inner((0.25 * Wk).T)                      # [128, 256]
    wvt_p = to_p_inner((0.25 * Wv).T)                      # [128, 256]
    wot_p = np.zeros((128, 256), f32)
    wot_p[:D + 1] = np.concatenate([Wo.T, bo_eff[None, :]], axis=0)
    wqt_p = to_p_inner(Wq.T)                                # [128, 128]
    wkvo = np.ascontiguousarray(
        np.concatenate([wkt_p, wvt_p, wot_p, wqt_p], axis=1)
    ).astype(bf16)                                          # [128, 896]

    bqd = np.ascontiguousarray(
        np.concatenate([bq, bq])[:, None]
    ).astype(f32)                                           # [128, 1]

    shared = {"wkvo": wkvo, "bqd": bqd}
    in_maps = []
    for b in range(B):
        m = dict(shared)
        m["x1"] = np.ascontiguousarray(x1[b].reshape(C1, HW))
        m["x2"] = np.ascontiguousarray(x2[b].reshape(C2, HW))
        in_maps.append(m)
    return in_maps


def run(inputs, trace=False, **trace_kwargs):
    from concourse.bass_utils import run_bass_kernel_spmd

    nc = _get_nc()
    in_maps = _prep_in_maps(**inputs)
    res = run_bass_kernel_spmd(
        nc, in_maps, list(range(B)), trace=trace, **trace_kwargs
    )
    out = np.stack([res.results[i]["out"] for i in range(B)])
    out = out.reshape(B, C1, H, W).astype(np.float32)
    return out, res


def kernel(**inputs) -> np.ndarray:
    out, _ = run(inputs, trace=False)
    return out


# revision 33
# speedup vs baseline: 1.2843x; 1.0957x over previous
"""Trainium2 Bass kernel for pooled cross-attention block (dense_transformer).

Reference computation per batch element b (B=8, one per NeuronCore):
  x2p = 2x2 mean-pool(x2)                      [512, 32, 32]
  Q = Wq @ x1  + bq                            [64, 4096]   (d-part layout)
  K = Wk @ x2p + bk                            [64, 1024]
  V = Wv @ x2p + bv                            [64, 1024]
  attn = softmax_n(Q^T K)                      [4096, 1024]
  out  = attn @ V^T                            [4096, 64]
  y    = out @ Wo^T + bo -> [256, 4096] ; result = x1 + y

Kernel strategy (all on-chip per core, streamed over n in 512-col chunks):
  - scores computed TRANSPOSED: sT[m, n] = K^T Q so softmax's reduce dim m
    is the partition dim; the row-sum r[n] is obtained for free by
    augmenting V^T with a ones column (row 64 of the U = V_aug^T expS
    accumulation).  No PE transposes anywhere.
  - bias algebra (all exact): bk drops (per-row softmax shift invariance);
    bq folded into Q on the PSUM->SBUF copy (DVE tensor_scalar);
    bv folded into bo' = bo + Wo@bv on host (attn rows sum to 1);
    bo' enters via the ones-row of the normalized U against an augmented
    Wo^T.
  - K and Q are produced twice (column-group-tiled matmuls run concurrently
    in the PE array, so the duplicate is ~free) so the scores matmuls can be
    issued as row-group-tiled CONCURRENT pairs: mi even uses array rows
    0-63 / K copy 1 / Q copy 1, mi odd uses rows 64-127 / the duplicates.
    Each pair lands in one 2-bank-wide PSUM tile, consumed by a single
    1024-wide exp ACTIVATE.
  - PE clock: the HAM activity monitor keeps the PE at 1.2 GHz unless it
    sees sustained back-to-back matmul activity.  A dense warmup burst at
    t=0 (during the input DMAs) plus a trickle of dummy matmuls between
    the DMA-paced phase-A bursts keeps the array at 2.4 GHz.
  - 2x2 pooling: two strided DVE adds (bf16); the 1/4 scale is folded into
    Wk/Wv on the host.
  - softmax normalization: 1/r via the fast custom-DVE reciprocal
    (~18 bits), broadcast on GpSimd, applied on DVE; the output projection
    matmuls for chunk j are interleaved into chunk j+1's PE stream so the
    PE never waits on the normalization tail.
"""

import sys

for _p in ("/opt/trn_rl_repo",):
    if _p not in sys.path:
        sys.path.insert(0, _p)

import numpy as np

B, C1, C2, H, W, D = 8, 256, 512, 64, 64, 64
HW = H * W            # n (query) size: 4096
M = (H // 2) * (W // 2)  # kv size: 1024
NCH = 512             # n-chunk (one fp32 PSUM bank)
NCHUNKS = HW // NCH   # 8
C1T = C1 // 128       # 2
C2T = C2 // 128       # 4
MT = M // 128         # 8

WARMUP_MMS = 16       # dense burst at t=0 (crosses the ~3.4us HAM window)
TRICKLE_MMS = 8       # dummy MMs after each phase-A group to keep HAM warm

DBG = False           # add intermediate-dump outputs (debugging only)

_CACHE = {}


def _build():
    import concourse.bass as bass
    import concourse.tile as tile
    from concourse import bacc, mybir

    dt = mybir.dt
    f32, bf16, f32r = dt.float32, dt.bfloat16, dt.float32r
    Exp = mybir.ActivationFunctionType.Exp

    nc = bacc.Bacc(
        "TRN2", target_bir_lowering=False, debug=False, num_devices=8
    )
    x1 = nc.dram_tensor("x1", [C1, HW], f32r, kind="ExternalInput").ap()
    x2 = nc.dram_tensor("x2", [C2, HW], f32, kind="ExternalInput").ap()
    # packed weights: one bf16 blob + one f32 blob -> 2 DMAs total
    wkvo = nc.dram_tensor("wkvo", [128, 768], bf16, kind="ExternalInput").ap()
    wqbq = nc.dram_tensor("wqbq", [128, 129], f32r, kind="ExternalInput").ap()
    out = nc.dram_tensor("out", [C1, HW], f32, kind="ExternalOutput").ap()
    if DBG:
        d_ksb = nc.dram_tensor("d_ksb", [128, M], bf16, kind="ExternalOutput").ap()
        d_qall = nc.dram_tensor(
            "d_qall", [128, NCHUNKS, NCH], bf16, kind="ExternalOutput"
        ).ap()
        d_es = nc.dram_tensor(
            "d_es", [128, 4, 2 * NCH], bf16, kind="ExternalOutput"
        ).ap()
        d_on = nc.dram_tensor("d_on", [D + 1, NCH], bf16, kind="ExternalOutput").ap()
        d_rinv = nc.dram_tensor("d_rinv", [1, NCH], f32, kind="ExternalOutput").ap()
        d_r = nc.dram_tensor("d_r", [1, NCH], f32, kind="ExternalOutput").ap()


    x1v = x1.rearrange("(t p) n -> p t n", p=128)    # [128, 2, HW]
    outv = out.rearrange("(t p) n -> p t n", p=128)  # [128, 2, HW]

    from contextlib import ExitStack

    with tile.TileContext(nc) as tc, ExitStack() as ctx:
        pool = lambda name, bufs, **kw: ctx.enter_context(
            tc.tile_pool(name=name, bufs=bufs, **kw)
        )
        consts = pool("consts", 1)
        warm = pool("warm", 1)
        x2st = pool("x2st", 4)
        x1p = pool("x1p", 8)
        poolp = pool("poolp", 2)
        sbfp = pool("sbfp", 2)
        kvsb = pool("kvsb", 1)
        qallp = pool("qallp", 1)
        esp = pool("esp", 3)
        rp = pool("rp", 2)
        rbp = pool("rbp", 2)
        onp = pool("onp", 2)
        youtp = pool("youtp", 4)
        ps_s = pool("ps_s", 2, space="PSUM")   # [128,1024] wide: 2 banks x 2
        ps_u = pool("ps_u", 2, space="PSUM")   # 1 bank x 2
        ps_q = pool("ps_q", 1, space="PSUM")   # 1 bank
        ps_y = pool("ps_y", 1, space="PSUM")   # 1 bank

        # ---- t=0: preload exp table, warmup tiles -----------------------
        actw = warm.tile([1, 8], f32, tag="actw")
        nc.vector.memset(actw, 0.0)
        actw2 = warm.tile([1, 8], f32, tag="actw2")
        nc.scalar.activation(actw2, actw, Exp)

        wu_w = warm.tile([128, 64], bf16, tag="wu_w")
        nc.vector.memset(wu_w, 0.0)
        wu_x = warm.tile([128, NCH], bf16, tag="wu_x")
        nc.vector.memset(wu_x, 0.0)

        # ---- input DMAs: x2 first (4 x 2MiB) ---------------------------
        x2t = [None] * C2T
        for ci in range(C2T):
            x2t[ci] = x2st.tile([128, HW], f32, tag="x2t", name=f"x2t{ci}")
            nc.sync.dma_start(
                out=x2t[ci], in_=x2[ci * 128:(ci + 1) * 128, :]
            )

        # ---- constants (2 DMAs), then all of x1 (8) --------------------
        wkvo_sb = consts.tile([128, 768], bf16, tag="wkvo")
        nc.sync.dma_start(out=wkvo_sb, in_=wkvo)
        wqbq_sb = consts.tile([128, 129], f32r, tag="wqbq")
        nc.sync.dma_start(out=wqbq_sb, in_=wqbq)
        wkt_sb = wkvo_sb[:, 0:256].rearrange("p (c d) -> p c d", c=C2T)
        wvt_sb = wkvo_sb[:, 256:512].rearrange("p (c d) -> p c d", c=C2T)
        wot_sb = wkvo_sb[:, 512:768]              # [65 used, 256]
        wqt_sb = wqbq_sb[:, 0:128].rearrange("p (t d) -> p t d", t=C1T)
        bq_sb = wqbq_sb[:, 128:129].bitcast(f32)  # [128, 1]
        x1t = [None] * NCHUNKS
        for nj in range(NCHUNKS):
            x1t[nj] = x1p.tile(
                [128, C1T, NCH], f32r, tag="x1t", name=f"x1t{nj}"
            )
            nc.sync.dma_start(
                out=x1t[nj], in_=x1v[:, :, nj * NCH:(nj + 1) * NCH]
            )

        # ---- PE warmup burst (keeps HAM at 2.4 GHz) --------------------
        wu_ps = ps_y.tile([128, NCH], f32, tag="y", name="wu_ps")
        for i in range(WARMUP_MMS):
            nc.tensor.matmul(
                wu_ps[0:64, :], lhsT=wu_w, rhs=wu_x, start=True, stop=True
            )

        # ---- phase A: pool x2, project K (x2 col-dup) and V^T ----------
        k_pack = ps_s.tile([128, 2 * NCH], f32, tag="s", name="k_pack")
        v_ps = ps_u.tile([128, MT, D], f32, tag="u", name="v_ps")
        for ci in range(C2T):
            x2v = x2t[ci].rearrange(
                "p (h w2 two) -> p h w2 two", w2=W // 2, two=2
            )
            t1 = poolp.tile([128, H, W // 2], bf16, tag="t1", name="t1")
            nc.vector.tensor_add(t1, x2v[:, :, :, 0], x2v[:, :, :, 1])
            t1v = t1.rearrange("p (h2 two) w2 -> p h2 two w2", two=2)
            s_bf = sbfp.tile([128, M], bf16, tag="s", name="s_bf")
            s3 = s_bf.rearrange("p (h2 w2) -> p h2 w2", h2=H // 2)
            nc.vector.tensor_add(s3, t1v[:, :, 0, :], t1v[:, :, 1, :])
            first, last = ci == 0, ci == C2T - 1
            for h in range(2):
                rhs = s_bf[:, h * NCH:(h + 1) * NCH]
                # col-group pair: rows 0-63 and 64-127 of k_pack run
                # concurrently in the PE array
                nc.tensor.matmul(
                    k_pack[0:64, h * NCH:(h + 1) * NCH],
                    lhsT=wkt_sb[:, ci, :], rhs=rhs, start=first, stop=last,
                    skip_group_check=True,
                )
                nc.tensor.matmul(
                    k_pack[64:128, h * NCH:(h + 1) * NCH],
                    lhsT=wkt_sb[:, ci, :], rhs=rhs, start=first, stop=last,
                    skip_group_check=True,
                )
            for mj in range(MT):
                nc.tensor.matmul(
                    v_ps[:, mj, :],
                    lhsT=s_bf[:, mj * 128:(mj + 1) * 128],
                    rhs=wvt_sb[:, ci, :],
                    start=first, stop=last,
                    skip_group_check=True,
                )
            # trickle: keep the PE busy across the DMA-paced gap
            for i in range(TRICKLE_MMS):
                nc.tensor.matmul(
                    wu_ps[0:64, :], lhsT=wu_w, rhs=s_bf[:, 0:NCH],
                    start=True, stop=True,
                )
        k_sb = kvsb.tile([128, M], bf16, tag="ksb")
        nc.vector.tensor_copy(k_sb, k_pack)
        v_aug = kvsb.tile([128, MT, D + 1], bf16, tag="vaug")
        nc.vector.memset(v_aug[:, :, D], 1.0)
        nc.vector.tensor_copy(v_aug[:, :, 0:D], v_ps)

        # ---- Q projection (fp32; rows 64-127 dup'd via SBUF DMA) -------
        q_all = qallp.tile([128, NCHUNKS, NCH], bf16, tag="qall")

        def q_proj(nj):
            q_ps = ps_q.tile([64, NCH], f32, tag="q", name=f"q_ps{nj}")
            for t in range(C1T):
                nc.tensor.matmul(
                    q_ps, lhsT=wqt_sb[:, t, :], rhs=x1t[nj][:, t, :],
                    start=(t == 0), stop=(t == C1T - 1),
                )
            nc.vector.tensor_scalar_add(
                q_all[0:64, nj, :], q_ps, bq_sb[0:64, :]
            )
            nc.scalar.dma_start(
                out=q_all[64:128, nj, :], in_=q_all[0:64, nj, :]
            )

        q_proj(0)
        q_proj(1)

        # ---- phase B: stream n-chunks ----------------------------------
        # PE stream per chunk (software-pipelined so the PE never waits on
        # exp): sp0 sp1 up0 sp2 up1 sp3 [y0'] up2 [y1'] up3, where y' are
        # the previous chunk's output projections.
        pend = None  # (u_ps, nj) awaiting tail + output projection

        def tail_begin(p):
            # normalization tail of the previous chunk (DVE/GpSimd queues)
            u_prev, njp = p
            r_sb = rp.tile([1, NCH], f32, tag="rsb", name="r_sb")
            nc.vector.tensor_copy(r_sb, u_prev[D:D + 1, :])
            rinv = rp.tile([1, NCH], f32, tag="rinv", name="rinv")
            nc.vector.reciprocal_approx_fast(rinv, r_sb)
            if DBG and njp == 0:
                nc.sync.dma_start(out=d_r, in_=r_sb)
                nc.sync.dma_start(out=d_rinv, in_=rinv)
            rb = rbp.tile([D + 1, NCH], f32, tag="rb", name="rb")
            nc.gpsimd.partition_broadcast(rb, rinv)
            on = onp.tile([D + 1, NCH], bf16, tag="on", name="on")
            nc.vector.tensor_mul(on, u_prev, rb)
            if DBG and njp == 0:
                nc.sync.dma_start(out=d_on, in_=on)
            yo = youtp.tile([128, C1T, NCH], f32, tag="yo", name="yo")
            return on, yo, njp

        def tail_y(t, on, yo, njp):
            y_ps = ps_y.tile([128, NCH], f32, tag="y", name=f"y_ps{njp}_{t}")
            nc.tensor.matmul(
                y_ps, lhsT=wot_sb[0:D + 1, t * 128:(t + 1) * 128], rhs=on,
                start=True, stop=True,
            )
            nc.vector.tensor_add(yo[:, t, :], x1t[njp][:, t, :].bitcast(f32), y_ps)

        def tail_flush(on, yo, njp):
            nc.sync.dma_start(
                out=outv[:, :, njp * NCH:(njp + 1) * NCH], in_=yo
            )

        for nj in range(NCHUNKS):
            if nj + 2 < NCHUNKS:
                q_proj(nj + 2)
            u_ps = ps_u.tile([D + 1, NCH], f32, tag="u", name=f"u_ps{nj}")
            tl = tail_begin(pend) if pend is not None else None

            s_w = [None] * 4
            es = [None] * 4

            def s_pair(p):
                s_w[p] = ps_s.tile([128, 2 * NCH], f32, tag="s", name=f"s_w{nj}_{p}")
                mi_a, mi_b = 2 * p, 2 * p + 1
                nc.tensor.matmul(
                    s_w[p][:, 0:NCH],
                    lhsT=k_sb[0:64, mi_a * 128:(mi_a + 1) * 128],
                    rhs=q_all[0:64, nj, :], start=True, stop=True,
                )
                nc.tensor.matmul(
                    s_w[p][:, NCH:2 * NCH],
                    lhsT=k_sb[64:128, mi_b * 128:(mi_b + 1) * 128],
                    rhs=q_all[64:128, nj, :], start=True, stop=True,
                )
                es[p] = esp.tile([128, 2 * NCH], bf16, tag="es", name=f"es{nj}_{p}")
                nc.scalar.activation(es[p], s_w[p], Exp)
                if DBG and nj == 0:
                    nc.sync.dma_start(out=d_es[:, p, :], in_=es[p])

            def u_pair(p):
                mi_a, mi_b = 2 * p, 2 * p + 1
                nc.tensor.matmul(
                    u_ps, lhsT=v_aug[:, mi_a, :], rhs=es[p][:, 0:NCH],
                    start=(p == 0), stop=False,
                )
                nc.tensor.matmul(
                    u_ps, lhsT=v_aug[:, mi_b, :], rhs=es[p][:, NCH:2 * NCH],
                    start=False, stop=(p == 3),
                )

            s_pair(0)
            s_pair(1)
            u_pair(0)
            s_pair(2)
            u_pair(1)
            s_pair(3)
            if tl is not None:
                tail_y(0, *tl)
            u_pair(2)
            if tl is not None:
                tail_y(1, *tl)
                tail_flush(*tl)
            u_pair(3)
            pend = (u_ps, nj)

        tl = tail_begin(pend)
        tail_y(0, *tl)
        tail_y(1, *tl)
        tail_flush(*tl)
        if DBG:
            nc.sync.dma_start(out=d_ksb, in_=k_sb)
            nc.sync.dma_start(out=d_qall, in_=q_all)
            nc.sync.dma_start(out=d_x1b, in_=x1b[0])
    nc.compile()
    return nc


def _get_nc():
    if "nc" not in _CACHE:
        _CACHE["nc"] = _build()
    return _CACHE["nc"]


def _prep_in_maps(x1, x2, Wq, bq, Wk, bk, Wv, bv, Wo, bo):
    import ml_dtypes

    bf16 = ml_dtypes.bfloat16
    f32 = np.float32
    x1 = np.asarray(x1, f32)
    x2 = np.asarray(x2, f32)
    Wq = np.asarray(Wq, f32)
    Wk = np.asarray(Wk, f32)
    Wv = np.asarray(Wv, f32)
    Wo = np.asarray(Wo, f32)
    bq = np.asarray(bq, f32)
    bk = np.asarray(bk, f32)
    bv = np.asarray(bv, f32)
    bo = np.asarray(bo, f32)

    # bk is softmax-invariant (constant per score row) and is dropped.
    # bv folds into the output bias because attention rows sum to one.
    bo_eff = bo + Wo @ bv

    def to_p_inner(w):  # [CT*128, D] -> [128, CT*D]
        ct = w.shape[0] // 128
        return np.ascontiguousarray(
            w.reshape(ct, 128, D).transpose(1, 0, 2).reshape(128, ct * D)
        )

    wkt_p = to_p_inner((0.25 * Wk).T)                      # [128, 256]
    wvt_p = to_p_inner((0.25 * Wv).T)                      # [128, 256]
    wot_p = np.zeros((128, 256), f32)
    wot_p[:D + 1] = np.concatenate([Wo.T, bo_eff[None, :]], axis=0)
    wkvo = np.ascontiguousarray(
        np.concatenate([wkt_p, wvt_p, wot_p], axis=1)
    ).astype(bf16)                                          # [128, 768]

    wqt_p = to_p_inner(Wq.T)                                # [128, 128]
    bq_dup = np.concatenate([bq, bq])[:, None]              # [128, 1]
    wqbq = np.ascontiguousarray(
        np.concatenate([wqt_p, bq_dup], axis=1)
    ).astype(f32)                                           # [128, 129]

    shared = {"wkvo": wkvo, "wqbq": wqbq}
    in_maps = []
    for b in range(B):
        m = dict(shared)
        m["x1"] = np.ascontiguousarray(x1[b].reshape(C1, HW))
        m["x2"] = np.ascontiguousarray(x2[b].reshape(C2, HW))
        in_maps.append(m)
    return in_maps


def run(inputs, trace=False, **trace_kwargs):
    from concourse.bass_utils import run_bass_kernel_spmd

    nc = _get_nc()
    in_maps = _prep_in_maps(**inputs)
    res = run_bass_kernel_spmd(
        nc, in_maps, list(range(B)), trace=trace, **trace_kwargs
    )
    out = np.stack([res.results[i]["out"] for i in range(B)])
    out = out.reshape(B, C1, H, W).astype(np.float32)
    return out, res


def kernel(**inputs) -> np.ndarray:
    out, _ = run(inputs, trace=False)
    return out


# revision 39
# speedup vs baseline: 1.4604x; 1.1371x over previous
"""Trainium2 Bass kernel for pooled cross-attention block (dense_transformer).

Reference computation per batch element b (B=8, one per NeuronCore):
  x2p = 2x2 mean-pool(x2)                      [512, 32, 32]
  Q = Wq @ x1  + bq                            [64, 4096]   (d-part layout)
  K = Wk @ x2p + bk                            [64, 1024]
  V = Wv @ x2p + bv                            [64, 1024]
  attn = softmax_n(Q^T K)                      [4096, 1024]
  out  = attn @ V^T                            [4096, 64]
  y    = out @ Wo^T + bo -> [256, 4096] ; result = x1 + y

Kernel strategy (all on-chip per core, streamed over n in 512-col chunks):
  - scores computed TRANSPOSED: sT[m, n] = K^T Q so softmax's reduce dim m
    is the partition dim; the row-sum r[n] is obtained for free by
    augmenting V^T with a ones column (row 64 of the U = V_aug^T expS
    accumulation).  No PE transposes anywhere.
  - bias algebra (all exact): bk drops (per-row softmax shift invariance);
    bq folded into Q on the PSUM->SBUF copy (DVE tensor_scalar);
    bv folded into bo' = bo + Wo@bv on host (attn rows sum to 1);
    bo' enters via the ones-row of the normalized U against an augmented
    Wo^T.
  - K and Q are produced twice (column-group-tiled matmuls run concurrently
    in the PE array, so the duplicate is ~free) so the scores matmuls can be
    issued as row-group-tiled CONCURRENT pairs: mi even uses array rows
    0-63 / K copy 1 / Q copy 1, mi odd uses rows 64-127 / the duplicates.
    Each pair lands in one 2-bank-wide PSUM tile, consumed by a single
    1024-wide exp ACTIVATE.
  - PE clock: the HAM activity monitor keeps the PE at 1.2 GHz unless it
    sees sustained back-to-back matmul activity.  A dense warmup burst at
    t=0 (during the input DMAs) plus a trickle of dummy matmuls between
    the DMA-paced phase-A bursts keeps the array at 2.4 GHz.
  - 2x2 pooling: two strided DVE adds (bf16); the 1/4 scale is folded into
    Wk/Wv on the host.
  - softmax normalization: 1/r via the fast custom-DVE reciprocal
    (~18 bits), broadcast on GpSimd, applied on DVE; the output projection
    matmuls for chunk j are interleaved into chunk j+1's PE stream so the
    PE never waits on the normalization tail.
"""

import sys

for _p in ("/opt/trn_rl_repo",):
    if _p not in sys.path:
        sys.path.insert(0, _p)

import numpy as np

B, C1, C2, H, W, D = 8, 256, 512, 64, 64, 64
HW = H * W            # n (query) size: 4096
M = (H // 2) * (W // 2)  # kv size: 1024
NCH = 512             # n-chunk (one fp32 PSUM bank)
NCHUNKS = HW // NCH   # 8
C1T = C1 // 128       # 2
C2T = C2 // 128       # 4
MT = M // 128         # 8

DBG = False           # add intermediate-dump outputs (debugging only)

_CACHE = {}


def _build():
    import concourse.bass as bass
    import concourse.tile as tile
    from concourse import bacc, mybir

    dt = mybir.dt
    f32, bf16, f32r = dt.float32, dt.bfloat16, dt.float32r
    Exp = mybir.ActivationFunctionType.Exp

    nc = bacc.Bacc(
        "TRN2", target_bir_lowering=False, debug=False, num_devices=8
    )
    x1 = nc.dram_tensor("x1", [C1, HW], f32, kind="ExternalInput").ap()
    x2 = nc.dram_tensor("x2", [C2, HW], f32, kind="ExternalInput").ap()
    # packed weights: one bf16 blob + one f32 blob -> 2 DMAs total
    wkvo = nc.dram_tensor("wkvo", [128, 896], bf16, kind="ExternalInput").ap()
    bqd = nc.dram_tensor("bqd", [128, 1], f32, kind="ExternalInput").ap()
    out = nc.dram_tensor("out", [C1, HW], f32, kind="ExternalOutput").ap()
    if DBG:
        d_ksb = nc.dram_tensor("d_ksb", [128, M], bf16, kind="ExternalOutput").ap()
        d_qall = nc.dram_tensor(
            "d_qall", [128, NCHUNKS, NCH], bf16, kind="ExternalOutput"
        ).ap()
        d_es = nc.dram_tensor(
            "d_es", [128, 4, 2 * NCH], bf16, kind="ExternalOutput"
        ).ap()
        d_on = nc.dram_tensor("d_on", [D + 1, NCH], bf16, kind="ExternalOutput").ap()
        d_rinv = nc.dram_tensor("d_rinv", [1, NCH], f32, kind="ExternalOutput").ap()
        d_r = nc.dram_tensor("d_r", [1, NCH], f32, kind="ExternalOutput").ap()


    x1v = x1.rearrange("(t p) n -> p t n", p=128)    # [128, 2, HW]
    outv = out.rearrange("(t p) n -> p t n", p=128)  # [128, 2, HW]

    from contextlib import ExitStack

    with tile.TileContext(nc) as tc, ExitStack() as ctx:
        pool = lambda name, bufs, **kw: ctx.enter_context(
            tc.tile_pool(name=name, bufs=bufs, **kw)
        )
        consts = pool("consts", 1)
        warm = pool("warm", 1)
        x2st = pool("x2st", 8)
        x1p = pool("x1p", 8)
        poolp = pool("poolp", 2)
        sbfp = pool("sbfp", 2)
        kvsb = pool("kvsb", 1)
        qallp = pool("qallp", 1)
        esp = pool("esp", 3)
        rp = pool("rp", 2)
        rbp = pool("rbp", 2)
        onp = pool("onp", 2)
        youtp = pool("youtp", 4)
        ps_s = pool("ps_s", 2, space="PSUM")   # [128,1024] wide: 2 banks x 2
        ps_u = pool("ps_u", 2, space="PSUM")   # 1 bank x 2
        ps_q = pool("ps_q", 1, space="PSUM")   # 1 bank
        ps_y = pool("ps_y", 1, space="PSUM")   # 1 bank

        # ---- t=0: preload exp table ------------------------------------
        actw = warm.tile([1, 8], f32, tag="actw")
        nc.vector.memset(actw, 0.0)
        actw2 = warm.tile([1, 8], f32, tag="actw2")
        nc.scalar.activation(actw2, actw, Exp)

        # ---- input DMAs: x2 first, 8 halves ordered (hi, ci) -----------
        HHW = HW // 2
        x2t = {}
        for hi in range(2):
            for ci in range(C2T):
                t = x2st.tile([128, HHW], f32, tag="x2t", name=f"x2t{hi}{ci}")
                x2t[hi, ci] = t
                nc.sync.dma_start(
                    out=t,
                    in_=x2[ci * 128:(ci + 1) * 128, hi * HHW:(hi + 1) * HHW],
                )

        # ---- constants (2 DMAs) ----------------------------------------
        wkvo_sb = consts.tile([128, 896], bf16, tag="wkvo")
        nc.sync.dma_start(out=wkvo_sb, in_=wkvo)
        bq_sb = consts.tile([128, 1], f32, tag="bqd")
        nc.sync.dma_start(out=bq_sb, in_=bqd)
        wkt_sb = wkvo_sb[:, 0:256].rearrange("p (c d) -> p c d", c=C2T)
        wvt_sb = wkvo_sb[:, 256:512].rearrange("p (c d) -> p c d", c=C2T)
        wot_sb = wkvo_sb[:, 512:768]              # [65 used, 256]
        wqt_sb = wkvo_sb[:, 768:896].rearrange("p (t d) -> p t d", t=C1T)

        # ---- x1: SWDGE casting DMAs (fp32->bf16 in the DMA datapath),
        # gated behind the 6th x2 transfer so they don't steal x2's HBM
        # bandwidth on the critical path.
        gate = warm.tile([1, 1], f32, tag="gate")
        nc.gpsimd.tensor_copy(gate, x2t[1, 1][0:1, 0:1])
        x1t = [None] * NCHUNKS
        for nj in range(NCHUNKS):
            x1t[nj] = x1p.tile(
                [128, C1T, NCH], bf16, tag="x1t", name=f"x1t{nj}"
            )
            nc.gpsimd.dma_start(
                out=x1t[nj], in_=x1v[:, :, nj * NCH:(nj + 1) * NCH]
            )

        # ---- phase A: pool x2, project K (col-dup pairs) and V^T -------
        k_pack = ps_s.tile([128, 2 * NCH], f32, tag="s", name="k_pack")
        v_ps = ps_u.tile([128, MT, D], f32, tag="u", name="v_ps")
        k_sb = kvsb.tile([128, M], bf16, tag="ksb")
        for hi in range(2):
            for ci in range(C2T):
                x2v = x2t[hi, ci].rearrange(
                    "p (h w2 two) -> p h w2 two", w2=W // 2, two=2
                )
                t1 = poolp.tile([128, H // 2, W // 2], bf16, tag="t1", name="t1")
                nc.vector.tensor_add(t1, x2v[:, :, :, 0], x2v[:, :, :, 1])
                t1v = t1.rearrange("p (h2 two) w2 -> p h2 two w2", two=2)
                s_bf = sbfp.tile([128, NCH], bf16, tag="s", name="s_bf")
                s3 = s_bf.rearrange("p (h2 w2) -> p h2 w2", h2=H // 4)
                nc.vector.tensor_add(s3, t1v[:, :, 0, :], t1v[:, :, 1, :])
                first, last = ci == 0, ci == C2T - 1
                # col-group pair: rows 0-63 / 64-127 concurrently
                nc.tensor.matmul(
                    k_pack[0:64, hi * NCH:(hi + 1) * NCH],
                    lhsT=wkt_sb[:, ci, :], rhs=s_bf, start=first, stop=last,
                    skip_group_check=True,
                )
                nc.tensor.matmul(
                    k_pack[64:128, hi * NCH:(hi + 1) * NCH],
                    lhsT=wkt_sb[:, ci, :], rhs=s_bf, start=first, stop=last,
                    skip_group_check=True,
                )
                for mj in range(MT // 2):
                    nc.tensor.matmul(
                        v_ps[:, hi * (MT // 2) + mj, :],
                        lhsT=s_bf[:, mj * 128:(mj + 1) * 128],
                        rhs=wvt_sb[:, ci, :],
                        start=first, stop=last,
                        skip_group_check=True,
                    )
            nc.vector.tensor_copy(
                k_sb[:, hi * NCH:(hi + 1) * NCH],
                k_pack[:, hi * NCH:(hi + 1) * NCH],
            )
        v_aug = kvsb.tile([128, MT, D + 1], bf16, tag="vaug")
        nc.vector.memset(v_aug[:, :, D], 1.0)
        nc.vector.tensor_copy(v_aug[:, :, 0:D], v_ps)

        # ---- Q projection (bf16 col-dup pairs) -------------------------
        q_all = qallp.tile([128, NCHUNKS, NCH], bf16, tag="qall")

        def q_proj(nj):
            q_ps = ps_q.tile([128, NCH], f32, tag="q", name=f"q_ps{nj}")
            for t in range(C1T):
                nc.tensor.matmul(
                    q_ps[0:64, :], lhsT=wqt_sb[:, t, :], rhs=x1t[nj][:, t, :],
                    start=(t == 0), stop=(t == C1T - 1),
                    skip_group_check=True,
                )
                nc.tensor.matmul(
                    q_ps[64:128, :], lhsT=wqt_sb[:, t, :], rhs=x1t[nj][:, t, :],
                    start=(t == 0), stop=(t == C1T - 1),
                    skip_group_check=True,
                )
            nc.vector.tensor_scalar_add(q_all[:, nj, :], q_ps, bq_sb)

        q_proj(0)
        q_proj(1)

        # ---- phase B: stream n-chunks ----------------------------------
        # PE stream per chunk (software-pipelined so the PE never waits on
        # exp): sp0 sp1 up0 sp2 up1 sp3 [y0'] up2 [y1'] up3, where y' are
        # the previous chunk's output projections.
        pend = None  # (u_ps, nj) awaiting tail + output projection

        def tail_begin(p):
            # normalization tail of the previous chunk (DVE/GpSimd queues)
            u_prev, njp = p
            r_sb = rp.tile([1, NCH], f32, tag="rsb", name="r_sb")
            nc.vector.tensor_copy(r_sb, u_prev[D:D + 1, :])
            rinv = rp.tile([1, NCH], f32, tag="rinv", name="rinv")
            nc.vector.reciprocal_approx_fast(rinv, r_sb)
            if DBG and njp == 0:
                nc.sync.dma_start(out=d_r, in_=r_sb)
                nc.sync.dma_start(out=d_rinv, in_=rinv)
            rb = rbp.tile([D + 1, NCH], f32, tag="rb", name="rb")
            nc.gpsimd.partition_broadcast(rb, rinv)
            on = onp.tile([D + 1, NCH], bf16, tag="on", name="on")
            nc.vector.tensor_mul(on, u_prev, rb)
            if DBG and njp == 0:
                nc.sync.dma_start(out=d_on, in_=on)
            yo = youtp.tile([128, C1T, NCH], f32, tag="yo", name="yo")
            return on, yo, njp

        def tail_y(t, on, yo, njp):
            y_ps = ps_y.tile([128, NCH], f32, tag="y", name=f"y_ps{njp}_{t}")
            nc.tensor.matmul(
                y_ps, lhsT=wot_sb[0:D + 1, t * 128:(t + 1) * 128], rhs=on,
                start=True, stop=True,
            )
            nc.vector.tensor_add(yo[:, t, :], x1t[njp][:, t, :], y_ps)
            nc.sync.dma_start(
                out=outv[:, t, njp * NCH:(njp + 1) * NCH], in_=yo[:, t, :]
            )

        def tail_flush(on, yo, njp):
            pass

        for nj in range(NCHUNKS):
            if nj + 2 < NCHUNKS:
                q_proj(nj + 2)
            u_ps = ps_u.tile([D + 1, NCH], f32, tag="u", name=f"u_ps{nj}")
            tl = tail_begin(pend) if pend is not None else None

            s_w = [None] * 4
            es = [None] * 4

            def s_pair(p):
                s_w[p] = ps_s.tile([128, 2 * NCH], f32, tag="s", name=f"s_w{nj}_{p}")
                mi_a, mi_b = 2 * p, 2 * p + 1
                nc.tensor.matmul(
                    s_w[p][:, 0:NCH],
                    lhsT=k_sb[0:64, mi_a * 128:(mi_a + 1) * 128],
                    rhs=q_all[0:64, nj, :], start=True, stop=True,
                )
                nc.tensor.matmul(
                    s_w[p][:, NCH:2 * NCH],
                    lhsT=k_sb[64:128, mi_b * 128:(mi_b + 1) * 128],
                    rhs=q_all[64:128, nj, :], start=True, stop=True,
                )
                es[p] = esp.tile([128, 2 * NCH], bf16, tag="es", name=f"es{nj}_{p}")
                nc.scalar.activation(es[p], s_w[p], Exp)
                if DBG and nj == 0:
                    nc.sync.dma_start(out=d_es[:, p, :], in_=es[p])

            def u_pair(p):
                mi_a, mi_b = 2 * p, 2 * p + 1
                nc.tensor.matmul(
                    u_ps, lhsT=v_aug[:, mi_a, :], rhs=es[p][:, 0:NCH],
                    start=(p == 0), stop=False,
                )
                nc.tensor.matmul(
                    u_ps, lhsT=v_aug[:, mi_b, :], rhs=es[p][:, NCH:2 * NCH],
                    start=False, stop=(p == 3),
                )

            s_pair(0)
            s_pair(1)
            u_pair(0)
            s_pair(2)
            u_pair(1)
            s_pair(3)
            if tl is not None:
                tail_y(0, *tl)
            u_pair(2)
            if tl is not None:
                tail_y(1, *tl)
                tail_flush(*tl)
            u_pair(3)
            pend = (u_ps, nj)

        tl = tail_begin(pend)
        tail_y(0, *tl)
        tail_y(1, *tl)
        tail_flush(*tl)
        if DBG:
            nc.sync.dma_start(out=d_ksb, in_=k_sb)
            nc.sync.dma_start(out=d_qall, in_=q_all)
            nc.sync.dma_start(out=d_x1b, in_=x1b[0])
    nc.compile()
    return nc


def _get_nc():
    if "nc" not in _CACHE:
        _CACHE["nc"] = _build()
    return _CACHE["nc"]


def _prep_in_maps(x1, x2, Wq, bq, Wk, bk, Wv, bv, Wo, bo):
    import ml_dtypes

    bf16 = ml_dtypes.bfloat16
    f32 = np.float32
    x1 = np.asarray(x1, f32)
    x2 = np.asarray(x2, f32)
    Wq = np.asarray(Wq, f32)
    Wk = np.asarray(Wk, f32)
    Wv = np.asarray(Wv, f32)
    Wo = np.asarray(Wo, f32)
    bq = np.asarray(bq, f32)
    bk = np.asarray(bk, f32)
    bv = np.asarray(bv, f32)
    bo = np.asarray(bo, f32)

    # bk is softmax-invariant (constant per score row) and is dropped.
    # bv folds into the output bias because attention rows sum to one.
    bo_eff = bo + Wo @ bv

    def to_p_inner(w):  # [CT*128, D] -> [128, CT*D]
        ct = w.shape[0] // 128
        return np.ascontiguousarray(
            w.reshape(ct, 128, D).transpose(1, 0, 2).reshape(128, ct * D)
        )

    wkt_p = to_p_inner((0.25 * Wk).T)                      # [128, 256]
    wvt_p = to_p_inner((0.25 * Wv).T)                      # [128, 256]
    wot_p = np.zeros((128, 256), f32)
    wot_p[:D + 1] = np.concatenate([Wo.T, bo_eff[None, :]], axis=0)
    wqt_p = to_p_inner(Wq.T)                                # [128, 128]
    wkvo = np.ascontiguousarray(
        np.concatenate([wkt_p, wvt_p, wot_p, wqt_p], axis=1)
    ).astype(bf16)                                          # [128, 896]

    bqd = np.ascontiguousarray(
        np.concatenate([bq, bq])[:, None]
    ).astype(f32)                                           # [128, 1]

    shared = {"wkvo": wkvo, "bqd": bqd}
    in_maps = []
    for b in range(B):
        m = dict(shared)
        m["x1"] = np.ascontiguousarray(x1[b].reshape(C1, HW))
        m["x2"] = np.ascontiguousarray(x2[b].reshape(C2, HW))
        in_maps.append(m)
    return in_maps


def run(inputs, trace=False, **trace_kwargs):
    from concourse.bass_utils import run_bass_kernel_spmd

    nc = _get_nc()
    in_maps = _prep_in_maps(**inputs)
    res = run_bass_kernel_spmd(
        nc, in_maps, list(range(B)), trace=trace, **trace_kwargs
    )
    out = np.stack([res.results[i]["out"] for i in range(B)])
    out = out.reshape(B, C1, H, W).astype(np.float32)
    return out, res


def kernel(**inputs) -> np.ndarray:
    out, _ = run(inputs, trace=False)
    return out


# revision 41
# speedup vs baseline: 1.5048x; 1.0304x over previous
"""Trainium2 Bass kernel for pooled cross-attention block (dense_transformer).

Reference computation per batch element b (B=8, one per NeuronCore):
  x2p = 2x2 mean-pool(x2)                      [512, 32, 32]
  Q = Wq @ x1  + bq                            [64, 4096]   (d-part layout)
  K = Wk @ x2p + bk                            [64, 1024]
  V = Wv @ x2p + bv                            [64, 1024]
  attn = softmax_n(Q^T K)                      [4096, 1024]
  out  = attn @ V^T                            [4096, 64]
  y    = out @ Wo^T + bo -> [256, 4096] ; result = x1 + y

Kernel strategy (all on-chip per core, streamed over n in 512-col chunks):
  - scores computed TRANSPOSED: sT[m, n] = K^T Q so softmax's reduce dim m
    is the partition dim; the row-sum r[n] is obtained for free by
    augmenting V^T with a ones column (row 64 of the U = V_aug^T expS
    accumulation).  No PE transposes anywhere.
  - bias algebra (all exact): bk drops (per-row softmax shift invariance);
    bq folded into Q on the PSUM->SBUF copy (DVE tensor_scalar);
    bv folded into bo' = bo + Wo@bv on host (attn rows sum to 1);
    bo' enters via the ones-row of the normalized U against an augmented
    Wo^T.
  - K and Q are produced twice (column-group-tiled matmuls run concurrently
    in the PE array, so the duplicate is ~free) so the scores matmuls can be
    issued as row-group-tiled CONCURRENT pairs: mi even uses array rows
    0-63 / K copy 1 / Q copy 1, mi odd uses rows 64-127 / the duplicates.
    Each pair lands in one 2-bank-wide PSUM tile, consumed by a single
    1024-wide exp ACTIVATE.
  - PE clock: the HAM activity monitor keeps the PE at 1.2 GHz unless it
    sees sustained back-to-back matmul activity.  A dense warmup burst at
    t=0 (during the input DMAs) plus a trickle of dummy matmuls between
    the DMA-paced phase-A bursts keeps the array at 2.4 GHz.
  - 2x2 pooling: two strided DVE adds (bf16); the 1/4 scale is folded into
    Wk/Wv on the host.
  - softmax normalization: 1/r via the fast custom-DVE reciprocal
    (~18 bits), broadcast on GpSimd, applied on DVE; the output projection
    matmuls for chunk j are interleaved into chunk j+1's PE stream so the
    PE never waits on the normalization tail.
"""

import sys

for _p in ("/opt/trn_rl_repo",):
    if _p not in sys.path:
        sys.path.insert(0, _p)

import numpy as np

B, C1, C2, H, W, D = 8, 256, 512, 64, 64, 64
HW = H * W            # n (query) size: 4096
M = (H // 2) * (W // 2)  # kv size: 1024
NCH = 512             # n-chunk (one fp32 PSUM bank)
NCHUNKS = HW // NCH   # 8
C1T = C1 // 128       # 2
C2T = C2 // 128       # 4
MT = M // 128         # 8

DBG = False           # add intermediate-dump outputs (debugging only)

_CACHE = {}


def _build():
    import concourse.bass as bass
    import concourse.tile as tile
    from concourse import bacc, mybir

    dt = mybir.dt
    f32, bf16, f32r = dt.float32, dt.bfloat16, dt.float32r
    Exp = mybir.ActivationFunctionType.Exp

    nc = bacc.Bacc(
        "TRN2", target_bir_lowering=False, debug=False, num_devices=8
    )
    x1 = nc.dram_tensor("x1", [C1, HW], f32, kind="ExternalInput").ap()
    x2 = nc.dram_tensor("x2", [C2, HW], f32, kind="ExternalInput").ap()
    # packed weights: one bf16 blob + one f32 blob -> 2 DMAs total
    wkvo = nc.dram_tensor("wkvo", [128, 896], bf16, kind="ExternalInput").ap()
    bqd = nc.dram_tensor("bqd", [128, 1], f32, kind="ExternalInput").ap()
    out = nc.dram_tensor("out", [C1, HW], f32, kind="ExternalOutput").ap()
    if DBG:
        d_ksb = nc.dram_tensor("d_ksb", [128, M], bf16, kind="ExternalOutput").ap()
        d_qall = nc.dram_tensor(
            "d_qall", [128, NCHUNKS, NCH], bf16, kind="ExternalOutput"
        ).ap()
        d_es = nc.dram_tensor(
            "d_es", [128, 4, 2 * NCH], bf16, kind="ExternalOutput"
        ).ap()
        d_on = nc.dram_tensor("d_on", [D + 1, NCH], bf16, kind="ExternalOutput").ap()
        d_rinv = nc.dram_tensor("d_rinv", [1, NCH], f32, kind="ExternalOutput").ap()
        d_r = nc.dram_tensor("d_r", [1, NCH], f32, kind="ExternalOutput").ap()


    x1v = x1.rearrange("(t p) n -> p t n", p=128)    # [128, 2, HW]
    outv = out.rearrange("(t p) n -> p t n", p=128)  # [128, 2, HW]

    from contextlib import ExitStack

    with tile.TileContext(nc) as tc, ExitStack() as ctx:
        pool = lambda name, bufs, **kw: ctx.enter_context(
            tc.tile_pool(name=name, bufs=bufs, **kw)
        )
        consts = pool("consts", 1)
        warm = pool("warm", 1)
        x2st = pool("x2st", 8)
        x1p = pool("x1p", 8)
        poolp = pool("poolp", 2)
        sbfp = pool("sbfp", 2)
        kvsb = pool("kvsb", 1)
        qallp = pool("qallp", 1)
        esp = pool("esp", 3)
        rp = pool("rp", 2)
        rbp = pool("rbp", 2)
        onp = pool("onp", 2)
        youtp = pool("youtp", 4)
        ps_s = pool("ps_s", 2, space="PSUM")   # [128,1024] wide: 2 banks x 2
        ps_u = pool("ps_u", 2, space="PSUM")   # 1 bank x 2
        ps_q = pool("ps_q", 1, space="PSUM")   # 1 bank
        ps_y = pool("ps_y", 1, space="PSUM")   # 1 bank

        # ---- t=0: preload exp table ------------------------------------
        actw = warm.tile([1, 8], f32, tag="actw")
        nc.vector.memset(actw, 0.0)
        actw2 = warm.tile([1, 8], f32, tag="actw2")
        nc.scalar.activation(actw2, actw, Exp)

        # ---- input DMAs: x2 first, 8 halves ordered (hi, ci) -----------
        HHW = HW // 2
        x2t = {}
        for hi in range(2):
            for ci in range(C2T):
                t = x2st.tile([128, HHW], f32, tag="x2t", name=f"x2t{hi}{ci}")
                x2t[hi, ci] = t
                nc.sync.dma_start(
                    out=t,
                    in_=x2[ci * 128:(ci + 1) * 128, hi * HHW:(hi + 1) * HHW],
                )

        # ---- constants (2 DMAs) ----------------------------------------
        wkvo_sb = consts.tile([128, 896], bf16, tag="wkvo")
        nc.sync.dma_start(out=wkvo_sb, in_=wkvo)
        bq_sb = consts.tile([128, 1], f32, tag="bqd")
        nc.sync.dma_start(out=bq_sb, in_=bqd)
        wkt_sb = wkvo_sb[:, 0:256].rearrange("p (c d) -> p c d", c=C2T)
        wvt_sb = wkvo_sb[:, 256:512].rearrange("p (c d) -> p c d", c=C2T)
        wot_sb = wkvo_sb[:, 512:768]              # [65 used, 256]
        wqt_sb = wkvo_sb[:, 768:896].rearrange("p (t d) -> p t d", t=C1T)

        # ---- x1: SWDGE casting DMAs (fp32->bf16 in the DMA datapath),
        # gated behind the 6th x2 transfer so they don't steal x2's HBM
        # bandwidth on the critical path.  Tile schedules by data deps (not
        # program order), so the gate must be a real WAW dep: a tiny copy
        # of the gate value into each destination tile before its DMA.
        gate = warm.tile([1, 1], f32, tag="gate")
        nc.gpsimd.tensor_copy(gate, x2t[1, 1][0:1, 0:1])
        x1t = [None] * NCHUNKS
        for nj in range(NCHUNKS):
            x1t[nj] = x1p.tile(
                [128, C1T, NCH], bf16, tag="x1t", name=f"x1t{nj}"
            )
            nc.gpsimd.tensor_copy(x1t[nj][0:1, 0:1, 0:1], gate)
            nc.gpsimd.dma_start(
                out=x1t[nj], in_=x1v[:, :, nj * NCH:(nj + 1) * NCH]
            )

        # ---- phase A: pool x2, project K (col-dup pairs) and V^T -------
        k_pack = ps_s.tile([128, 2 * NCH], f32, tag="s", name="k_pack")
        v_ps = ps_u.tile([128, MT, D], f32, tag="u", name="v_ps")
        k_sb = kvsb.tile([128, M], bf16, tag="ksb")
        for hi in range(2):
            for ci in range(C2T):
                x2v = x2t[hi, ci].rearrange(
                    "p (h w2 two) -> p h w2 two", w2=W // 2, two=2
                )
                t1 = poolp.tile([128, H // 2, W // 2], bf16, tag="t1", name="t1")
                nc.vector.tensor_add(t1, x2v[:, :, :, 0], x2v[:, :, :, 1])
                t1v = t1.rearrange("p (h2 two) w2 -> p h2 two w2", two=2)
                s_bf = sbfp.tile([128, NCH], bf16, tag="s", name="s_bf")
                s3 = s_bf.rearrange("p (h2 w2) -> p h2 w2", h2=H // 4)
                nc.vector.tensor_add(s3, t1v[:, :, 0, :], t1v[:, :, 1, :])
                first, last = ci == 0, ci == C2T - 1
                # col-group pair: rows 0-63 / 64-127 concurrently
                nc.tensor.matmul(
                    k_pack[0:64, hi * NCH:(hi + 1) * NCH],
                    lhsT=wkt_sb[:, ci, :], rhs=s_bf, start=first, stop=last,
                    skip_group_check=True,
                )
                nc.tensor.matmul(
                    k_pack[64:128, hi * NCH:(hi + 1) * NCH],
                    lhsT=wkt_sb[:, ci, :], rhs=s_bf, start=first, stop=last,
                    skip_group_check=True,
                )
                for mj in range(MT // 2):
                    nc.tensor.matmul(
                        v_ps[:, hi * (MT // 2) + mj, :],
                        lhsT=s_bf[:, mj * 128:(mj + 1) * 128],
                        rhs=wvt_sb[:, ci, :],
                        start=first, stop=last,
                        skip_group_check=True,
                    )
            nc.vector.tensor_copy(
                k_sb[:, hi * NCH:(hi + 1) * NCH],
                k_pack[:, hi * NCH:(hi + 1) * NCH],
            )
        v_aug = kvsb.tile([128, MT, D + 1], bf16, tag="vaug")
        nc.vector.memset(v_aug[:, :, D], 1.0)
        nc.vector.tensor_copy(v_aug[:, :, 0:D], v_ps)

        # ---- Q projection (bf16 col-dup pairs) -------------------------
        q_all = qallp.tile([128, NCHUNKS, NCH], bf16, tag="qall")

        def q_proj(nj):
            q_ps = ps_q.tile([128, NCH], f32, tag="q", name=f"q_ps{nj}")
            for t in range(C1T):
                nc.tensor.matmul(
                    q_ps[0:64, :], lhsT=wqt_sb[:, t, :], rhs=x1t[nj][:, t, :],
                    start=(t == 0), stop=(t == C1T - 1),
                    skip_group_check=True,
                )
                nc.tensor.matmul(
                    q_ps[64:128, :], lhsT=wqt_sb[:, t, :], rhs=x1t[nj][:, t, :],
                    start=(t == 0), stop=(t == C1T - 1),
                    skip_group_check=True,
                )
            nc.vector.tensor_scalar_add(q_all[:, nj, :], q_ps, bq_sb)

        q_proj(0)
        q_proj(1)

        # ---- phase B: stream n-chunks ----------------------------------
        # PE stream per chunk (software-pipelined so the PE never waits on
        # exp): sp0 sp1 up0 sp2 up1 sp3 [y0'] up2 [y1'] up3, where y' are
        # the previous chunk's output projections.
        pend = None  # (u_ps, nj) awaiting tail + output projection

        def tail_begin(p):
            # normalization tail of the previous chunk (DVE/GpSimd queues)
            u_prev, njp = p
            r_sb = rp.tile([1, NCH], f32, tag="rsb", name="r_sb")
            nc.vector.tensor_copy(r_sb, u_prev[D:D + 1, :])
            rinv = rp.tile([1, NCH], f32, tag="rinv", name="rinv")
            nc.vector.reciprocal_approx_fast(rinv, r_sb)
            if DBG and njp == 0:
                nc.sync.dma_start(out=d_r, in_=r_sb)
                nc.sync.dma_start(out=d_rinv, in_=rinv)
            rb = rbp.tile([D + 1, NCH], f32, tag="rb", name="rb")
            nc.gpsimd.partition_broadcast(rb, rinv)
            on = onp.tile([D + 1, NCH], bf16, tag="on", name="on")
            nc.vector.tensor_mul(on, u_prev, rb)
            if DBG and njp == 0:
                nc.sync.dma_start(out=d_on, in_=on)
            yo = youtp.tile([128, C1T, NCH], f32, tag="yo", name="yo")
            return on, yo, njp

        def tail_y(t, on, yo, njp):
            y_ps = ps_y.tile([128, NCH], f32, tag="y", name=f"y_ps{njp}_{t}")
            nc.tensor.matmul(
                y_ps, lhsT=wot_sb[0:D + 1, t * 128:(t + 1) * 128], rhs=on,
                start=True, stop=True,
            )
            nc.vector.tensor_add(yo[:, t, :], x1t[njp][:, t, :], y_ps)
            nc.sync.dma_start(
                out=outv[:, t, njp * NCH:(njp + 1) * NCH], in_=yo[:, t, :]
            )

        def tail_flush(on, yo, njp):
            pass

        for nj in range(NCHUNKS):
            if nj + 2 < NCHUNKS:
                q_proj(nj + 2)
            u_ps = ps_u.tile([D + 1, NCH], f32, tag="u", name=f"u_ps{nj}")
            tl = tail_begin(pend) if pend is not None else None

            s_w = [None] * 4
            es = [None] * 4

            def s_pair(p):
                s_w[p] = ps_s.tile([128, 2 * NCH], f32, tag="s", name=f"s_w{nj}_{p}")
                mi_a, mi_b = 2 * p, 2 * p + 1
                nc.tensor.matmul(
                    s_w[p][:, 0:NCH],
                    lhsT=k_sb[0:64, mi_a * 128:(mi_a + 1) * 128],
                    rhs=q_all[0:64, nj, :], start=True, stop=True,
                )
                nc.tensor.matmul(
                    s_w[p][:, NCH:2 * NCH],
                    lhsT=k_sb[64:128, mi_b * 128:(mi_b + 1) * 128],
                    rhs=q_all[64:128, nj, :], start=True, stop=True,
                )
                es[p] = esp.tile([128, 2 * NCH], bf16, tag="es", name=f"es{nj}_{p}")
                nc.scalar.activation(es[p], s_w[p], Exp)
                if DBG and nj == 0:
                    nc.sync.dma_start(out=d_es[:, p, :], in_=es[p])

            def u_pair(p):
                mi_a, mi_b = 2 * p, 2 * p + 1
                nc.tensor.matmul(
                    u_ps, lhsT=v_aug[:, mi_a, :], rhs=es[p][:, 0:NCH],
                    start=(p == 0), stop=False,
                )
                nc.tensor.matmul(
                    u_ps, lhsT=v_aug[:, mi_b, :], rhs=es[p][:, NCH:2 * NCH],
                    start=False, stop=(p == 3),
                )

            s_pair(0)
            s_pair(1)
            u_pair(0)
            s_pair(2)
            u_pair(1)
            s_pair(3)
            if tl is not None:
                tail_y(0, *tl)
            u_pair(2)
            if tl is not None:
                tail_y(1, *tl)
                tail_flush(*tl)
            u_pair(3)
            pend = (u_ps, nj)

        # final chunk: its tail is fully exposed, so pipeline it in two
        # 256-col halves to shorten the serial normalize->project->store
        # chain at the end of the kernel.
        u_prev, njp = pend
        NH = NCH // 2
        r_h, rb_h, on_h = [None] * 2, [None] * 2, [None] * 2
        for hf in range(2):
            sl = slice(hf * NH, (hf + 1) * NH)
            r_sb = rp.tile([1, NH], f32, tag="rsb", name=f"rf{hf}")
            nc.vector.tensor_copy(r_sb, u_prev[D:D + 1, sl])
            rinv = rp.tile([1, NH], f32, tag="rinv", name=f"rif{hf}")
            nc.vector.reciprocal_approx_fast(rinv, r_sb)
            r_h[hf] = rinv
        for hf in range(2):
            rb = rbp.tile([D + 1, NH], f32, tag="rb", name=f"rbf{hf}")
            nc.gpsimd.partition_broadcast(rb, r_h[hf])
            rb_h[hf] = rb
        for hf in range(2):
            sl = slice(hf * NH, (hf + 1) * NH)
            on = onp.tile([D + 1, NH], bf16, tag="on", name=f"onf{hf}")
            nc.vector.tensor_mul(on, u_prev[:, sl], rb_h[hf])
            for t in range(C1T):
                y_ps = ps_y.tile([128, NH], f32, tag="y", name=f"yf{hf}_{t}")
                nc.tensor.matmul(
                    y_ps, lhsT=wot_sb[0:D + 1, t * 128:(t + 1) * 128], rhs=on,
                    start=True, stop=True,
                )
                yo = youtp.tile([128, NH], f32, tag="yo", name=f"yof{hf}_{t}")
                nc.vector.tensor_add(yo, x1t[njp][:, t, sl], y_ps)
                nc.sync.dma_start(
                    out=outv[:, t, njp * NCH + hf * NH:njp * NCH + (hf + 1) * NH],
                    in_=yo,
                )
        if DBG:
            nc.sync.dma_start(out=d_ksb, in_=k_sb)
            nc.sync.dma_start(out=d_qall, in_=q_all)
            nc.sync.dma_start(out=d_x1b, in_=x1b[0])
    nc.compile()
    return nc


def _get_nc():
    if "nc" not in _CACHE:
        _CACHE["nc"] = _build()
    return _CACHE["nc"]


def _prep_in_maps(x1, x2, Wq, bq, Wk, bk, Wv, bv, Wo, bo):
    import ml_dtypes

    bf16 = ml_dtypes.bfloat16
    f32 = np.float32
    x1 = np.asarray(x1, f32)
    x2 = np.asarray(x2, f32)
    Wq = np.asarray(Wq, f32)
    Wk = np.asarray(Wk, f32)
    Wv = np.asarray(Wv, f32)
    Wo = np.asarray(Wo, f32)
    bq = np.asarray(bq, f32)
    bk = np.asarray(bk, f32)
    bv = np.asarray(bv, f32)
    bo = np.asarray(bo, f32)

    # bk is softmax-invariant (constant per score row) and is dropped.
    # bv folds into the output bias because attention rows sum to one.
    bo_eff = bo + Wo @ bv

    def to_p_inner(w):  # [CT*128, D] -> [128, CT*D]
        ct = w.shape[0] // 128
        return np.ascontiguousarray(
            w.reshape(ct, 128, D).transpose(1, 0, 2).reshape(128, ct * D)
        )

    wkt_p = to_p_inner((0.25 * Wk).T)                      # [128, 256]
    wvt_p = to_p_inner((0.25 * Wv).T)                      # [128, 256]
    wot_p = np.zeros((128, 256), f32)
    wot_p[:D + 1] = np.concatenate([Wo.T, bo_eff[None, :]], axis=0)
    wqt_p = to_p_inner(Wq.T)                                # [128, 128]
    wkvo = np.ascontiguousarray(
        np.concatenate([wkt_p, wvt_p, wot_p, wqt_p], axis=1)
    ).astype(bf16)                                          # [128, 896]

    bqd = np.ascontiguousarray(
        np.concatenate([bq, bq])[:, None]
    ).astype(f32)                                           # [128, 1]

    shared = {"wkvo": wkvo, "bqd": bqd}
    in_maps = []
    for b in range(B):
        m = dict(shared)
        m["x1"] = np.ascontiguousarray(x1[b].reshape(C1, HW))
        m["x2"] = np.ascontiguousarray(x2[b].reshape(C2, HW))
        in_maps.append(m)
    return in_maps


def run(inputs, trace=False, **trace_kwargs):
    from concourse.bass_utils import run_bass_kernel_spmd

    nc = _get_nc()
    in_maps = _prep_in_maps(**inputs)
    res = run_bass_kernel_spmd(
        nc, in_maps, list(range(B)), trace=trace, **trace_kwargs
    )
    out = np.stack([res.results[i]["out"] for i in range(B)])
    out = out.reshape(B, C1, H, W).astype(np.float32)
    return out, res


def kernel(**inputs) -> np.ndarray:
    out, _ = run(inputs, trace=False)
    return out


# revision 44
# speedup vs baseline: 1.5444x; 1.0263x over previous
"""Trainium2 Bass kernel for pooled cross-attention block (dense_transformer).

Reference computation per batch element b (B=8, one per NeuronCore):
  x2p = 2x2 mean-pool(x2)                      [512, 32, 32]
  Q = Wq @ x1  + bq                            [64, 4096]   (d-part layout)
  K = Wk @ x2p + bk                            [64, 1024]
  V = Wv @ x2p + bv                            [64, 1024]
  attn = softmax_n(Q^T K)                      [4096, 1024]
  out  = attn @ V^T                            [4096, 64]
  y    = out @ Wo^T + bo -> [256, 4096] ; result = x1 + y

Kernel strategy (all on-chip per core, streamed over n in 512-col chunks):
  - scores computed TRANSPOSED: sT[m, n] = K^T Q so softmax's reduce dim m
    is the partition dim; the row-sum r[n] is obtained for free by
    augmenting V^T with a ones column (row 64 of the U = V_aug^T expS
    accumulation).  No PE transposes anywhere.
  - bias algebra (all exact): bk drops (per-row softmax shift invariance);
    bq folded into Q on the PSUM->SBUF copy (DVE tensor_scalar);
    bv folded into bo' = bo + Wo@bv on host (attn rows sum to 1);
    bo' enters via the ones-row of the normalized U against an augmented
    Wo^T.
  - K and Q are produced twice (column-group-tiled matmuls run concurrently
    in the PE array, so the duplicate is ~free) so the scores matmuls can be
    issued as row-group-tiled CONCURRENT pairs: mi even uses array rows
    0-63 / K copy 1 / Q copy 1, mi odd uses rows 64-127 / the duplicates.
    Each pair lands in one 2-bank-wide PSUM tile, consumed by a single
    1024-wide exp ACTIVATE.
  - PE clock: the HAM activity monitor keeps the PE at 1.2 GHz unless it
    sees sustained back-to-back matmul activity.  A dense warmup burst at
    t=0 (during the input DMAs) plus a trickle of dummy matmuls between
    the DMA-paced phase-A bursts keeps the array at 2.4 GHz.
  - 2x2 pooling: two strided DVE adds (bf16); the 1/4 scale is folded into
    Wk/Wv on the host.
  - softmax normalization: 1/r via the fast custom-DVE reciprocal
    (~18 bits), broadcast on GpSimd, applied on DVE; the output projection
    matmuls for chunk j are interleaved into chunk j+1's PE stream so the
    PE never waits on the normalization tail.
"""

import sys

for _p in ("/opt/trn_rl_repo",):
    if _p not in sys.path:
        sys.path.insert(0, _p)

import numpy as np

B, C1, C2, H, W, D = 8, 256, 512, 64, 64, 64
HW = H * W            # n (query) size: 4096
M = (H // 2) * (W // 2)  # kv size: 1024
NCH = 512             # n-chunk (one fp32 PSUM bank)
NCHUNKS = HW // NCH   # 8
C1T = C1 // 128       # 2
C2T = C2 // 128       # 4
MT = M // 128         # 8

DBG = False           # add intermediate-dump outputs (debugging only)

_CACHE = {}


def _build():
    import concourse.bass as bass
    import concourse.tile as tile
    from concourse import bacc, mybir

    dt = mybir.dt
    f32, bf16, f32r = dt.float32, dt.bfloat16, dt.float32r
    Exp = mybir.ActivationFunctionType.Exp

    nc = bacc.Bacc(
        "TRN2", target_bir_lowering=False, debug=False, num_devices=8
    )
    x1 = nc.dram_tensor("x1", [C1, HW], f32, kind="ExternalInput").ap()
    x2 = nc.dram_tensor("x2", [C2, HW], f32, kind="ExternalInput").ap()
    # packed weights: one bf16 blob + one f32 blob -> 2 DMAs total
    wkvo = nc.dram_tensor("wkvo", [128, 896], bf16, kind="ExternalInput").ap()
    bqd = nc.dram_tensor("bqd", [128, 1], f32, kind="ExternalInput").ap()
    out = nc.dram_tensor("out", [C1, HW], f32, kind="ExternalOutput").ap()
    if DBG:
        d_ksb = nc.dram_tensor("d_ksb", [128, M], bf16, kind="ExternalOutput").ap()
        d_qall = nc.dram_tensor(
            "d_qall", [128, NCHUNKS, NCH], bf16, kind="ExternalOutput"
        ).ap()
        d_es = nc.dram_tensor(
            "d_es", [128, 4, 2 * NCH], bf16, kind="ExternalOutput"
        ).ap()
        d_on = nc.dram_tensor("d_on", [D + 1, NCH], bf16, kind="ExternalOutput").ap()
        d_rinv = nc.dram_tensor("d_rinv", [1, NCH], f32, kind="ExternalOutput").ap()
        d_r = nc.dram_tensor("d_r", [1, NCH], f32, kind="ExternalOutput").ap()


    x1v = x1.rearrange("(t p) n -> p t n", p=128)    # [128, 2, HW]
    outv = out.rearrange("(t p) n -> p t n", p=128)  # [128, 2, HW]

    from contextlib import ExitStack

    with tile.TileContext(nc) as tc, ExitStack() as ctx:
        pool = lambda name, bufs, **kw: ctx.enter_context(
            tc.tile_pool(name=name, bufs=bufs, **kw)
        )
        consts = pool("consts", 1)
        warm = pool("warm", 1)
        x2st = pool("x2st", 8)
        x1p = pool("x1p", 8)
        poolp = pool("poolp", 2)
        sbfp = pool("sbfp", 2)
        kvsb = pool("kvsb", 1)
        qallp = pool("qallp", 1)
        esp = pool("esp", 3)
        rp = pool("rp", 2)
        rbp = pool("rbp", 2)
        onp = pool("onp", 2)
        youtp = pool("youtp", 4)
        ps_s = pool("ps_s", 2, space="PSUM")   # [128,1024] wide: 2 banks x 2
        ps_u = pool("ps_u", 2, space="PSUM")   # 1 bank x 2
        ps_q = pool("ps_q", 1, space="PSUM")   # 1 bank
        ps_y = pool("ps_y", 1, space="PSUM")   # 1 bank

        # ---- t=0: preload exp table ------------------------------------
        actw = warm.tile([1, 8], f32, tag="actw")
        nc.vector.memset(actw, 0.0)
        actw2 = warm.tile([1, 8], f32, tag="actw2")
        nc.scalar.activation(actw2, actw, Exp)

        # ---- input DMAs: x2 first, 8 halves ordered (hi, ci) -----------
        HHW = HW // 2
        x2t = {}
        for hi in range(2):
            for ci in range(C2T):
                t = x2st.tile([128, HHW], f32, tag="x2t", name=f"x2t{hi}{ci}")
                x2t[hi, ci] = t
                # first transfer via the Scalar HWDGE ring: that queue is
                # free at t=0, so x2 streaming starts ~2us earlier
                eng = nc.scalar if (hi, ci) == (0, 0) else nc.sync
                eng.dma_start(
                    out=t,
                    in_=x2[ci * 128:(ci + 1) * 128, hi * HHW:(hi + 1) * HHW],
                )

        # ---- constants (2 DMAs) ----------------------------------------
        wkvo_sb = consts.tile([128, 896], bf16, tag="wkvo")
        nc.sync.dma_start(out=wkvo_sb, in_=wkvo)
        bq_sb = consts.tile([128, 1], f32, tag="bqd")
        nc.sync.dma_start(out=bq_sb, in_=bqd)
        wkt_sb = wkvo_sb[:, 0:256].rearrange("p (c d) -> p c d", c=C2T)
        wvt_sb = wkvo_sb[:, 256:512].rearrange("p (c d) -> p c d", c=C2T)
        wot_sb = wkvo_sb[:, 512:768]              # [65 used, 256]
        wqt_sb = wkvo_sb[:, 768:896].rearrange("p (t d) -> p t d", t=C1T)

        # ---- x1: SWDGE casting DMAs (fp32->bf16 in the DMA datapath),
        # gated behind the 6th x2 transfer so they don't steal x2's HBM
        # bandwidth on the critical path.  Tile schedules by data deps (not
        # program order), so the gate must be a real WAW dep: a tiny copy
        # of the gate value into each destination tile before its DMA.
        gate = warm.tile([1, 1], f32, tag="gate")
        nc.gpsimd.tensor_copy(gate, x2t[1, 3][0:1, 0:1])
        x1t = [None] * NCHUNKS
        for nj in range(NCHUNKS):
            x1t[nj] = x1p.tile(
                [128, C1T, NCH], bf16, tag="x1t", name=f"x1t{nj}"
            )
            nc.gpsimd.tensor_copy(x1t[nj][0:1, 0:1, 0:1], gate)
            nc.gpsimd.dma_start(
                out=x1t[nj], in_=x1v[:, :, nj * NCH:(nj + 1) * NCH]
            )

        # ---- phase A: pool x2, project K (col-dup pairs) and V^T -------
        k_pack = ps_s.tile([128, 2 * NCH], f32, tag="s", name="k_pack")
        v_ps = ps_u.tile([128, MT, D], f32, tag="u", name="v_ps")
        k_sb = kvsb.tile([128, M], bf16, tag="ksb")
        for hi in range(2):
            for ci in range(C2T):
                x2v = x2t[hi, ci].rearrange(
                    "p (h w2 two) -> p h w2 two", w2=W // 2, two=2
                )
                t1 = poolp.tile([128, H // 2, W // 2], bf16, tag="t1", name="t1")
                nc.vector.tensor_add(t1, x2v[:, :, :, 0], x2v[:, :, :, 1])
                t1v = t1.rearrange("p (h2 two) w2 -> p h2 two w2", two=2)
                s_bf = sbfp.tile([128, NCH], bf16, tag="s", name="s_bf")
                s3 = s_bf.rearrange("p (h2 w2) -> p h2 w2", h2=H // 4)
                nc.vector.tensor_add(s3, t1v[:, :, 0, :], t1v[:, :, 1, :])
                first, last = ci == 0, ci == C2T - 1
                # col-group pair: rows 0-63 / 64-127 concurrently
                nc.tensor.matmul(
                    k_pack[0:64, hi * NCH:(hi + 1) * NCH],
                    lhsT=wkt_sb[:, ci, :], rhs=s_bf, start=first, stop=last,
                    skip_group_check=True,
                )
                nc.tensor.matmul(
                    k_pack[64:128, hi * NCH:(hi + 1) * NCH],
                    lhsT=wkt_sb[:, ci, :], rhs=s_bf, start=first, stop=last,
                    skip_group_check=True,
                )
                for mj in range(MT // 2):
                    nc.tensor.matmul(
                        v_ps[:, hi * (MT // 2) + mj, :],
                        lhsT=s_bf[:, mj * 128:(mj + 1) * 128],
                        rhs=wvt_sb[:, ci, :],
                        start=first, stop=last,
                        skip_group_check=True,
                    )
            nc.vector.tensor_copy(
                k_sb[:, hi * NCH:(hi + 1) * NCH],
                k_pack[:, hi * NCH:(hi + 1) * NCH],
            )
        v_aug = kvsb.tile([128, MT, D + 1], bf16, tag="vaug")
        nc.vector.memset(v_aug[:, :, D], 1.0)
        nc.vector.tensor_copy(v_aug[:, :, 0:D], v_ps)

        # ---- Q projection (bf16 col-dup pairs) -------------------------
        q_all = qallp.tile([128, NCHUNKS, NCH], bf16, tag="qall")

        def q_proj(nj):
            q_ps = ps_q.tile([128, NCH], f32, tag="q", name=f"q_ps{nj}")
            for t in range(C1T):
                nc.tensor.matmul(
                    q_ps[0:64, :], lhsT=wqt_sb[:, t, :], rhs=x1t[nj][:, t, :],
                    start=(t == 0), stop=(t == C1T - 1),
                    skip_group_check=True,
                )
                nc.tensor.matmul(
                    q_ps[64:128, :], lhsT=wqt_sb[:, t, :], rhs=x1t[nj][:, t, :],
                    start=(t == 0), stop=(t == C1T - 1),
                    skip_group_check=True,
                )
            nc.vector.tensor_scalar_add(q_all[:, nj, :], q_ps, bq_sb)

        q_proj(0)
        q_proj(1)

        # ---- phase B: stream n-chunks ----------------------------------
        # PE stream per chunk (software-pipelined so the PE never waits on
        # exp): sp0 sp1 up0 sp2 up1 sp3 [y0'] up2 [y1'] up3, where y' are
        # the previous chunk's output projections.
        pend = None  # (u_ps, nj) awaiting tail + output projection

        def tail_begin(p):
            # normalization tail of the previous chunk (DVE/GpSimd queues)
            u_prev, njp = p
            r_sb = rp.tile([1, NCH], f32, tag="rsb", name="r_sb")
            nc.vector.tensor_copy(r_sb, u_prev[D:D + 1, :])
            rinv = rp.tile([1, NCH], f32, tag="rinv", name="rinv")
            nc.vector.reciprocal_approx_fast(rinv, r_sb)
            if DBG and njp == 0:
                nc.sync.dma_start(out=d_r, in_=r_sb)
                nc.sync.dma_start(out=d_rinv, in_=rinv)
            rb = rbp.tile([D + 1, NCH], f32, tag="rb", name="rb")
            nc.gpsimd.partition_broadcast(rb, rinv)
            on = onp.tile([D + 1, NCH], bf16, tag="on", name="on")
            nc.vector.tensor_mul(on, u_prev, rb)
            if DBG and njp == 0:
                nc.sync.dma_start(out=d_on, in_=on)
            yo = youtp.tile([128, C1T, NCH], f32, tag="yo", name="yo")
            return on, yo, njp

        def tail_y(t, on, yo, njp):
            y_ps = ps_y.tile([128, NCH], f32, tag="y", name=f"y_ps{njp}_{t}")
            nc.tensor.matmul(
                y_ps, lhsT=wot_sb[0:D + 1, t * 128:(t + 1) * 128], rhs=on,
                start=True, stop=True,
            )
            nc.vector.tensor_add(yo[:, t, :], x1t[njp][:, t, :], y_ps)
            nc.sync.dma_start(
                out=outv[:, t, njp * NCH:(njp + 1) * NCH], in_=yo[:, t, :]
            )

        def tail_flush(on, yo, njp):
            pass

        for nj in range(NCHUNKS):
            if nj + 2 < NCHUNKS:
                q_proj(nj + 2)
            u_ps = ps_u.tile([D + 1, NCH], f32, tag="u", name=f"u_ps{nj}")
            tl = tail_begin(pend) if pend is not None else None

            s_w = [None] * 4
            es = [None] * 4

            def s_pair(p):
                s_w[p] = ps_s.tile([128, 2 * NCH], f32, tag="s", name=f"s_w{nj}_{p}")
                mi_a, mi_b = 2 * p, 2 * p + 1
                nc.tensor.matmul(
                    s_w[p][:, 0:NCH],
                    lhsT=k_sb[0:64, mi_a * 128:(mi_a + 1) * 128],
                    rhs=q_all[0:64, nj, :], start=True, stop=True,
                )
                nc.tensor.matmul(
                    s_w[p][:, NCH:2 * NCH],
                    lhsT=k_sb[64:128, mi_b * 128:(mi_b + 1) * 128],
                    rhs=q_all[64:128, nj, :], start=True, stop=True,
                )
                es[p] = esp.tile([128, 2 * NCH], bf16, tag="es", name=f"es{nj}_{p}")
                nc.scalar.activation(es[p], s_w[p], Exp)
                if DBG and nj == 0:
                    nc.sync.dma_start(out=d_es[:, p, :], in_=es[p])

            def u_pair(p):
                mi_a, mi_b = 2 * p, 2 * p + 1
                nc.tensor.matmul(
                    u_ps, lhsT=v_aug[:, mi_a, :], rhs=es[p][:, 0:NCH],
                    start=(p == 0), stop=False,
                )
                nc.tensor.matmul(
                    u_ps, lhsT=v_aug[:, mi_b, :], rhs=es[p][:, NCH:2 * NCH],
                    start=False, stop=(p == 3),
                )

            s_pair(0)
            s_pair(1)
            u_pair(0)
            s_pair(2)
            u_pair(1)
            s_pair(3)
            if tl is not None:
                tail_y(0, *tl)
            u_pair(2)
            if tl is not None:
                tail_y(1, *tl)
                tail_flush(*tl)
            u_pair(3)
            pend = (u_ps, nj)

        # final chunk: its tail is fully exposed, so pipeline it in two
        # 256-col halves to shorten the serial normalize->project->store
        # chain at the end of the kernel.
        u_prev, njp = pend
        NH = NCH // 2
        r_h, rb_h, on_h = [None] * 2, [None] * 2, [None] * 2
        for hf in range(2):
            sl = slice(hf * NH, (hf + 1) * NH)
            r_sb = rp.tile([1, NH], f32, tag="rsb", name=f"rf{hf}")
            nc.vector.tensor_copy(r_sb, u_prev[D:D + 1, sl])
            rinv = rp.tile([1, NH], f32, tag="rinv", name=f"rif{hf}")
            nc.vector.reciprocal_approx_fast(rinv, r_sb)
            r_h[hf] = rinv
        for hf in range(2):
            rb = rbp.tile([D + 1, NH], f32, tag="rb", name=f"rbf{hf}")
            nc.gpsimd.partition_broadcast(rb, r_h[hf])
            rb_h[hf] = rb
        for hf in range(2):
            sl = slice(hf * NH, (hf + 1) * NH)
            on = onp.tile([D + 1, NH], bf16, tag="on", name=f"onf{hf}")
            nc.vector.tensor_mul(on, u_prev[:, sl], rb_h[hf])
            for t in range(C1T):
                # alternate PSUM slots (s-pool is idle by now) so the four
                # final projections overlap instead of serializing on WAR
                yp_pool = ps_y if (hf + t) % 2 == 0 else ps_s
                yp_tag = "y" if (hf + t) % 2 == 0 else "s"
                y_ps = yp_pool.tile(
                    [128, NH], f32, tag=yp_tag, name=f"yf{hf}_{t}"
                )
                nc.tensor.matmul(
                    y_ps, lhsT=wot_sb[0:D + 1, t * 128:(t + 1) * 128], rhs=on,
                    start=True, stop=True,
                )
                yo = youtp.tile([128, NH], f32, tag="yo", name=f"yof{hf}_{t}")
                nc.vector.tensor_add(yo, x1t[njp][:, t, sl], y_ps)
                nc.sync.dma_start(
                    out=outv[:, t, njp * NCH + hf * NH:njp * NCH + (hf + 1) * NH],
                    in_=yo,
                )
        if DBG:
            nc.sync.dma_start(out=d_ksb, in_=k_sb)
            nc.sync.dma_start(out=d_qall, in_=q_all)
            nc.sync.dma_start(out=d_x1b, in_=x1b[0])
    nc.compile()
    return nc


def _get_nc():
    if "nc" not in _CACHE:
        _CACHE["nc"] = _build()
    return _CACHE["nc"]


def _prep_in_maps(x1, x2, Wq, bq, Wk, bk, Wv, bv, Wo, bo):
    import ml_dtypes

    bf16 = ml_dtypes.bfloat16
    f32 = np.float32
    x1 = np.asarray(x1, f32)
    x2 = np.asarray(x2, f32)
    Wq = np.asarray(Wq, f32)
    Wk = np.asarray(Wk, f32)
    Wv = np.asarray(Wv, f32)
    Wo = np.asarray(Wo, f32)
    bq = np.asarray(bq, f32)
    bk = np.asarray(bk, f32)
    bv = np.asarray(bv, f32)
    bo = np.asarray(bo, f32)

    # bk is softmax-invariant (constant per score row) and is dropped.
    # bv folds into the output bias because attention rows sum to one.
    bo_eff = bo + Wo @ bv

    def to_p_inner(w):  # [CT*128, D] -> [128, CT*D]
        ct = w.shape[0] // 128
        return np.ascontiguousarray(
            w.reshape(ct, 128, D).transpose(1, 0, 2).reshape(128, ct * D)
        )

    wkt_p = to_p_inner((0.25 * Wk).T)                      # [128, 256]
    wvt_p = to_p_inner((0.25 * Wv).T)                      # [128, 256]
    wot_p = np.zeros((128, 256), f32)
    wot_p[:D + 1] = np.concatenate([Wo.T, bo_eff[None, :]], axis=0)
    wqt_p = to_p_inner(Wq.T)                                # [128, 128]
    wkvo = np.ascontiguousarray(
        np.concatenate([wkt_p, wvt_p, wot_p, wqt_p], axis=1)
    ).astype(bf16)                                          # [128, 896]

    bqd = np.ascontiguousarray(
        np.concatenate([bq, bq])[:, None]
    ).astype(f32)                                           # [128, 1]

    shared = {"wkvo": wkvo, "bqd": bqd}
    in_maps = []
    for b in range(B):
        m = dict(shared)
        m["x1"] = np.ascontiguousarray(x1[b].reshape(C1, HW))
        m["x2"] = np.ascontiguousarray(x2[b].reshape(C2, HW))
        in_maps.append(m)
    return in_maps


def run(inputs, trace=False, **trace_kwargs):
    from concourse.bass_utils import run_bass_kernel_spmd

    nc = _get_nc()
    in_maps = _prep_in_maps(**inputs)
    res = run_bass_kernel_spmd(
        nc, in_maps, list(range(B)), trace=trace, **trace_kwargs
    )
    out = np.stack([res.results[i]["out"] for i in range(B)])
    out = out.reshape(B, C1, H, W).astype(np.float32)
    return out, res


def kernel(**inputs) -> np.ndarray:
    out, _ = run(inputs, trace=False)
    return out


# revision 49
# speedup vs baseline: 1.5755x; 1.0201x over previous
"""Trainium2 Bass kernel for pooled cross-attention block (dense_transformer).

Reference computation per batch element b (B=8, one per NeuronCore):
  x2p = 2x2 mean-pool(x2)                      [512, 32, 32]
  Q = Wq @ x1  + bq                            [64, 4096]   (d-part layout)
  K = Wk @ x2p + bk                            [64, 1024]
  V = Wv @ x2p + bv                            [64, 1024]
  attn = softmax_n(Q^T K)                      [4096, 1024]
  out  = attn @ V^T                            [4096, 64]
  y    = out @ Wo^T + bo -> [256, 4096] ; result = x1 + y

Kernel strategy (all on-chip per core, streamed over n in 512-col chunks):
  - scores computed TRANSPOSED: sT[m, n] = K^T Q so softmax's reduce dim m
    is the partition dim; the row-sum r[n] is obtained for free by
    augmenting V^T with a ones column (row 64 of the U = V_aug^T expS
    accumulation).  No PE transposes anywhere.
  - bias algebra (all exact): bk drops (per-row softmax shift invariance);
    bq folded into Q on the PSUM->SBUF copy (DVE tensor_scalar);
    bv folded into bo' = bo + Wo@bv on host (attn rows sum to 1);
    bo' enters via the ones-row of the normalized U against an augmented
    Wo^T.
  - K and Q are produced twice (column-group-tiled matmuls run concurrently
    in the PE array, so the duplicate is ~free) so the scores matmuls can be
    issued as row-group-tiled CONCURRENT pairs: mi even uses array rows
    0-63 / K copy 1 / Q copy 1, mi odd uses rows 64-127 / the duplicates.
    Each pair lands in one 2-bank-wide PSUM tile, consumed by a single
    1024-wide exp ACTIVATE.
  - PE clock: the HAM activity monitor keeps the PE at 1.2 GHz unless it
    sees sustained back-to-back matmul activity.  A dense warmup burst at
    t=0 (during the input DMAs) plus a trickle of dummy matmuls between
    the DMA-paced phase-A bursts keeps the array at 2.4 GHz.
  - 2x2 pooling: two strided DVE adds (bf16); the 1/4 scale is folded into
    Wk/Wv on the host.
  - softmax normalization: 1/r via the fast custom-DVE reciprocal
    (~18 bits), broadcast on GpSimd, applied on DVE; the output projection
    matmuls for chunk j are interleaved into chunk j+1's PE stream so the
    PE never waits on the normalization tail.
"""

import sys

for _p in ("/opt/trn_rl_repo",):
    if _p not in sys.path:
        sys.path.insert(0, _p)

import numpy as np

B, C1, C2, H, W, D = 8, 256, 512, 64, 64, 64
HW = H * W            # n (query) size: 4096
M = (H // 2) * (W // 2)  # kv size: 1024
NCH = 512             # n-chunk (one fp32 PSUM bank)
NCHUNKS = HW // NCH   # 8
C1T = C1 // 128       # 2
C2T = C2 // 128       # 4
MT = M // 128         # 8

DBG = False           # add intermediate-dump outputs (debugging only)

_CACHE = {}


def _build():
    import concourse.bass as bass
    import concourse.tile as tile
    from concourse import bacc, mybir

    dt = mybir.dt
    f32, bf16, f32r = dt.float32, dt.bfloat16, dt.float32r
    Exp = mybir.ActivationFunctionType.Exp

    nc = bacc.Bacc(
        "TRN2", target_bir_lowering=False, debug=False, num_devices=8
    )
    x1 = nc.dram_tensor("x1", [C1, HW], f32, kind="ExternalInput").ap()
    x2 = nc.dram_tensor("x2", [C2, HW], f32, kind="ExternalInput").ap()
    # packed weights: one bf16 blob + one f32 blob -> 2 DMAs total
    wkvo = nc.dram_tensor("wkvo", [128, 896], bf16, kind="ExternalInput").ap()
    bqd = nc.dram_tensor("bqd", [128, 1], f32, kind="ExternalInput").ap()
    out = nc.dram_tensor("out", [C1, HW], f32, kind="ExternalOutput").ap()
    if DBG:
        d_ksb = nc.dram_tensor("d_ksb", [128, M], bf16, kind="ExternalOutput").ap()
        d_qall = nc.dram_tensor(
            "d_qall", [128, NCHUNKS, NCH], bf16, kind="ExternalOutput"
        ).ap()
        d_es = nc.dram_tensor(
            "d_es", [128, 4, 2 * NCH], bf16, kind="ExternalOutput"
        ).ap()
        d_on = nc.dram_tensor("d_on", [D + 1, NCH], bf16, kind="ExternalOutput").ap()
        d_rinv = nc.dram_tensor("d_rinv", [1, NCH], f32, kind="ExternalOutput").ap()
        d_r = nc.dram_tensor("d_r", [1, NCH], f32, kind="ExternalOutput").ap()


    x1v = x1.rearrange("(t p) n -> p t n", p=128)    # [128, 2, HW]
    outv = out.rearrange("(t p) n -> p t n", p=128)  # [128, 2, HW]

    from contextlib import ExitStack

    with tile.TileContext(nc) as tc, ExitStack() as ctx:
        pool = lambda name, bufs, **kw: ctx.enter_context(
            tc.tile_pool(name=name, bufs=bufs, **kw)
        )
        consts = pool("consts", 1)
        warm = pool("warm", 1)
        x2st = pool("x2st", 8)
        x1p = pool("x1p", 8)
        poolp = pool("poolp", 2)
        sbfp = pool("sbfp", 2)
        kvsb = pool("kvsb", 1)
        qallp = pool("qallp", 1)
        esp = pool("esp", 3)
        rp = pool("rp", 2)
        rbp = pool("rbp", 2)
        onp = pool("onp", 2)
        youtp = pool("youtp", 4)
        ps_s = pool("ps_s", 2, space="PSUM")   # [128,1024] wide: 2 banks x 2
        ps_u = pool("ps_u", 2, space="PSUM")   # 1 bank x 2
        ps_qy = pool("ps_qy", 2, space="PSUM")  # 1 bank x 2 (q + y rotate)
        ps_q = ps_y = ps_qy

        # ---- t=0: preload exp table ------------------------------------
        actw = warm.tile([1, 8], f32, tag="actw")
        nc.vector.memset(actw, 0.0)
        actw2 = warm.tile([1, 8], f32, tag="actw2")
        nc.scalar.activation(actw2, actw, Exp)

        # ---- input DMAs: x2 first, 8 halves ordered (hi, ci) -----------
        HHW = HW // 2
        x2t = {}
        for hi in range(2):
            for ci in range(C2T):
                t = x2st.tile([128, HHW], f32, tag="x2t", name=f"x2t{hi}{ci}")
                x2t[hi, ci] = t
                # first transfer via the Scalar HWDGE ring: that queue is
                # free at t=0, so x2 streaming starts ~2us earlier
                eng = nc.scalar if (hi, ci) == (0, 0) else nc.sync
                eng.dma_start(
                    out=t,
                    in_=x2[ci * 128:(ci + 1) * 128, hi * HHW:(hi + 1) * HHW],
                )

        # ---- constants (2 DMAs) ----------------------------------------
        wkvo_sb = consts.tile([128, 896], bf16, tag="wkvo")
        nc.sync.dma_start(out=wkvo_sb, in_=wkvo)
        bq_sb = consts.tile([128, 1], f32, tag="bqd")
        nc.sync.dma_start(out=bq_sb, in_=bqd)
        wkt_sb = wkvo_sb[:, 0:256].rearrange("p (c d) -> p c d", c=C2T)
        wvt_sb = wkvo_sb[:, 256:512].rearrange("p (c d) -> p c d", c=C2T)
        wot_sb = wkvo_sb[:, 512:768]              # [65 used, 256]
        wqt_sb = wkvo_sb[:, 768:896].rearrange("p (t d) -> p t d", t=C1T)

        # ---- x1: SWDGE casting DMAs (fp32->bf16 in the DMA datapath),
        # gated behind the 6th x2 transfer so they don't steal x2's HBM
        # bandwidth on the critical path.  Tile schedules by data deps (not
        # program order), so the gate must be a real WAW dep: a tiny copy
        # of the gate value into each destination tile before its DMA.
        gate_a = warm.tile([1, 1], f32, tag="gate_a")
        nc.gpsimd.tensor_copy(gate_a, x2t[1, 0][0:1, 0:1])
        gate_b = warm.tile([1, 1], f32, tag="gate_b")
        nc.gpsimd.tensor_copy(gate_b, x2t[1, 3][0:1, 0:1])
        x1t = [None] * NCHUNKS
        for nj in range(NCHUNKS):
            x1t[nj] = x1p.tile(
                [128, C1T, NCH], bf16, tag="x1t", name=f"x1t{nj}"
            )
            nc.gpsimd.tensor_copy(
                x1t[nj][0:1, 0:1, 0:1], gate_a if nj < 2 else gate_b
            )
            nc.gpsimd.dma_start(
                out=x1t[nj], in_=x1v[:, :, nj * NCH:(nj + 1) * NCH]
            )

        # ---- phase A: pool x2, project K (col-dup pairs) and V^T -------
        k_pack = ps_s.tile([128, 2 * NCH], f32, tag="s", name="k_pack")
        v_ps = ps_u.tile([128, MT, D], f32, tag="u", name="v_ps")
        k_sb = kvsb.tile([128, M], bf16, tag="ksb")
        for hi in range(2):
            for ci in range(C2T):
                x2v = x2t[hi, ci].rearrange(
                    "p (h w2 two) -> p h w2 two", w2=W // 2, two=2
                )
                t1 = poolp.tile([128, H // 2, W // 2], bf16, tag="t1", name="t1")
                nc.vector.tensor_add(t1, x2v[:, :, :, 0], x2v[:, :, :, 1])
                t1v = t1.rearrange("p (h2 two) w2 -> p h2 two w2", two=2)
                s_bf = sbfp.tile([128, NCH], bf16, tag="s", name="s_bf")
                s3 = s_bf.rearrange("p (h2 w2) -> p h2 w2", h2=H // 4)
                nc.vector.tensor_add(s3, t1v[:, :, 0, :], t1v[:, :, 1, :])
                first, last = ci == 0, ci == C2T - 1
                # col-group pair: rows 0-63 / 64-127 concurrently
                nc.tensor.matmul(
                    k_pack[0:64, hi * NCH:(hi + 1) * NCH],
                    lhsT=wkt_sb[:, ci, :], rhs=s_bf, start=first, stop=last,
                    skip_group_check=True,
                )
                nc.tensor.matmul(
                    k_pack[64:128, hi * NCH:(hi + 1) * NCH],
                    lhsT=wkt_sb[:, ci, :], rhs=s_bf, start=first, stop=last,
                    skip_group_check=True,
                )
                for mj in range(MT // 2):
                    nc.tensor.matmul(
                        v_ps[:, hi * (MT // 2) + mj, :],
                        lhsT=s_bf[:, mj * 128:(mj + 1) * 128],
                        rhs=wvt_sb[:, ci, :],
                        start=first, stop=last,
                        skip_group_check=True,
                    )
            nc.vector.tensor_copy(
                k_sb[:, hi * NCH:(hi + 1) * NCH],
                k_pack[:, hi * NCH:(hi + 1) * NCH],
            )
        v_aug = kvsb.tile([128, MT, D + 1], bf16, tag="vaug")
        nc.vector.memset(v_aug[:, :, D], 1.0)
        nc.vector.tensor_copy(v_aug[:, :, 0:D], v_ps)

        # ---- Q projection (bf16 col-dup pairs) -------------------------
        q_all = qallp.tile([128, NCHUNKS, NCH], bf16, tag="qall")

        def q_proj(nj):
            q_ps = ps_q.tile([128, NCH], f32, tag="qy", name=f"q_ps{nj}")
            for t in range(C1T):
                nc.tensor.matmul(
                    q_ps[0:64, :], lhsT=wqt_sb[:, t, :], rhs=x1t[nj][:, t, :],
                    start=(t == 0), stop=(t == C1T - 1),
                    skip_group_check=True,
                )
                nc.tensor.matmul(
                    q_ps[64:128, :], lhsT=wqt_sb[:, t, :], rhs=x1t[nj][:, t, :],
                    start=(t == 0), stop=(t == C1T - 1),
                    skip_group_check=True,
                )
            nc.vector.tensor_scalar_add(q_all[:, nj, :], q_ps, bq_sb)

        q_proj(0)
        q_proj(1)

        # ---- phase B: stream n-chunks ----------------------------------
        # PE stream per chunk (software-pipelined so the PE never waits on
        # exp): sp0 sp1 up0 sp2 up1 sp3 [y0'] up2 [y1'] up3, where y' are
        # the previous chunk's output projections.
        pend = None  # (u_ps, nj) awaiting tail + output projection

        def tail_begin(p):
            # normalization tail of the previous chunk (DVE/GpSimd queues)
            u_prev, njp = p
            r_sb = rp.tile([1, NCH], f32, tag="rsb", name="r_sb")
            nc.vector.tensor_copy(r_sb, u_prev[D:D + 1, :])
            rinv = rp.tile([1, NCH], f32, tag="rinv", name="rinv")
            nc.vector.reciprocal_approx_fast(rinv, r_sb)
            if DBG and njp == 0:
                nc.sync.dma_start(out=d_r, in_=r_sb)
                nc.sync.dma_start(out=d_rinv, in_=rinv)
            rb = rbp.tile([D + 1, NCH], f32, tag="rb", name="rb")
            nc.gpsimd.partition_broadcast(rb, rinv)
            on = onp.tile([D + 1, NCH], bf16, tag="on", name="on")
            nc.vector.tensor_mul(on, u_prev, rb)
            if DBG and njp == 0:
                nc.sync.dma_start(out=d_on, in_=on)
            yo = youtp.tile([128, C1T, NCH], f32, tag="yo", name="yo")
            return on, yo, njp

        def tail_y(t, on, yo, njp):
            y_ps = ps_y.tile([128, NCH], f32, tag="qy", name=f"y_ps{njp}_{t}")
            nc.tensor.matmul(
                y_ps, lhsT=wot_sb[0:D + 1, t * 128:(t + 1) * 128], rhs=on,
                start=True, stop=True,
            )
            nc.vector.tensor_add(yo[:, t, :], x1t[njp][:, t, :], y_ps)
            nc.sync.dma_start(
                out=outv[:, t, njp * NCH:(njp + 1) * NCH], in_=yo[:, t, :]
            )

        def tail_flush(on, yo, njp):
            pass

        for nj in range(NCHUNKS):
            if nj + 2 < NCHUNKS:
                q_proj(nj + 2)
            u_ps = ps_u.tile([D + 1, NCH], f32, tag="u", name=f"u_ps{nj}")
            tl = tail_begin(pend) if pend is not None else None

            s_w = [None] * 4
            es = [None] * 4

            def s_pair(p):
                s_w[p] = ps_s.tile([128, 2 * NCH], f32, tag="s", name=f"s_w{nj}_{p}")
                mi_a, mi_b = 2 * p, 2 * p + 1
                nc.tensor.matmul(
                    s_w[p][:, 0:NCH],
                    lhsT=k_sb[0:64, mi_a * 128:(mi_a + 1) * 128],
                    rhs=q_all[0:64, nj, :], start=True, stop=True,
                )
                nc.tensor.matmul(
                    s_w[p][:, NCH:2 * NCH],
                    lhsT=k_sb[64:128, mi_b * 128:(mi_b + 1) * 128],
                    rhs=q_all[64:128, nj, :], start=True, stop=True,
                )
                es[p] = esp.tile([128, 2 * NCH], bf16, tag="es", name=f"es{nj}_{p}")
                nc.scalar.activation(es[p], s_w[p], Exp)
                if DBG and nj == 0:
                    nc.sync.dma_start(out=d_es[:, p, :], in_=es[p])

            def u_pair(p):
                mi_a, mi_b = 2 * p, 2 * p + 1
                nc.tensor.matmul(
                    u_ps, lhsT=v_aug[:, mi_a, :], rhs=es[p][:, 0:NCH],
                    start=(p == 0), stop=False,
                )
                nc.tensor.matmul(
                    u_ps, lhsT=v_aug[:, mi_b, :], rhs=es[p][:, NCH:2 * NCH],
                    start=False, stop=(p == 3),
                )

            s_pair(0)
            s_pair(1)
            u_pair(0)
            s_pair(2)
            u_pair(1)
            s_pair(3)
            if tl is not None:
                tail_y(0, *tl)
            u_pair(2)
            if tl is not None:
                tail_y(1, *tl)
                tail_flush(*tl)
            u_pair(3)
            pend = (u_ps, nj)

        # final chunk: its tail is fully exposed, so pipeline it in two
        # 256-col halves to shorten the serial normalize->project->store
        # chain at the end of the kernel.
        u_prev, njp = pend
        NH = NCH // 2
        r_h, rb_h, on_h = [None] * 2, [None] * 2, [None] * 2
        for hf in range(2):
            sl = slice(hf * NH, (hf + 1) * NH)
            r_sb = rp.tile([1, NH], f32, tag="rsb", name=f"rf{hf}")
            nc.vector.tensor_copy(r_sb, u_prev[D:D + 1, sl])
            rinv = rp.tile([1, NH], f32, tag="rinv", name=f"rif{hf}")
            nc.vector.reciprocal_approx_fast(rinv, r_sb)
            r_h[hf] = rinv
        for hf in range(2):
            rb = rbp.tile([D + 1, NH], f32, tag="rb", name=f"rbf{hf}")
            nc.gpsimd.partition_broadcast(rb, r_h[hf])
            rb_h[hf] = rb
        for hf in range(2):
            sl = slice(hf * NH, (hf + 1) * NH)
            on = onp.tile([D + 1, NH], bf16, tag="on", name=f"onf{hf}")
            nc.vector.tensor_mul(on, u_prev[:, sl], rb_h[hf])
            for t in range(C1T):
                # alternate PSUM slots (s-pool is idle by now) so the four
                # final projections overlap instead of serializing on WAR
                yp_pool = ps_y if (hf + t) % 2 == 0 else ps_s
                yp_tag = "qy" if (hf + t) % 2 == 0 else "s"
                y_ps = yp_pool.tile(
                    [128, NH], f32, tag=yp_tag, name=f"yf{hf}_{t}"
                )
                nc.tensor.matmul(
                    y_ps, lhsT=wot_sb[0:D + 1, t * 128:(t + 1) * 128], rhs=on,
                    start=True, stop=True,
                )
                yo = youtp.tile([128, NH], f32, tag="yo", name=f"yof{hf}_{t}")
                nc.vector.tensor_add(yo, x1t[njp][:, t, sl], y_ps)
                nc.sync.dma_start(
                    out=outv[:, t, njp * NCH + hf * NH:njp * NCH + (hf + 1) * NH],
                    in_=yo,
                )
        if DBG:
            nc.sync.dma_start(out=d_ksb, in_=k_sb)
            nc.sync.dma_start(out=d_qall, in_=q_all)
            nc.sync.dma_start(out=d_x1b, in_=x1b[0])
    nc.compile()
    return nc


def _get_nc():
    if "nc" not in _CACHE:
        _CACHE["nc"] = _build()
    return _CACHE["nc"]


def _prep_in_maps(x1, x2, Wq, bq, Wk, bk, Wv, bv, Wo, bo):
    import ml_dtypes

    bf16 = ml_dtypes.bfloat16
    f32 = np.float32
    x1 = np.asarray(x1, f32)
    x2 = np.asarray(x2, f32)
    Wq = np.asarray(Wq, f32)
    Wk = np.asarray(Wk, f32)
    Wv = np.asarray(Wv, f32)
    Wo = np.asarray(Wo, f32)
    bq = np.asarray(bq, f32)
    bk = np.asarray(bk, f32)
    bv = np.asarray(bv, f32)
    bo = np.asarray(bo, f32)

    # bk is softmax-invariant (constant per score row) and is dropped.
    # bv folds into the output bias because attention rows sum to one.
    bo_eff = bo + Wo @ bv

    def to_p_inner(w):  # [CT*128, D] -> [128, CT*D]
        ct = w.shape[0] // 128
        return np.ascontiguousarray(
            w.reshape(ct, 128, D).transpose(1, 0, 2).reshape(128, ct * D)
        )

    wkt_p = to_p_inner((0.25 * Wk).T)                      # [128, 256]
    wvt_p = to_p_inner((0.25 * Wv).T)                      # [128, 256]
    wot_p = np.zeros((128, 256), f32)
    wot_p[:D + 1] = np.concatenate([Wo.T, bo_eff[None, :]], axis=0)
    wqt_p = to_p_inner(Wq.T)                                # [128, 128]
    wkvo = np.ascontiguousarray(
        np.concatenate([wkt_p, wvt_p, wot_p, wqt_p], axis=1)
    ).astype(bf16)                                          # [128, 896]

    bqd = np.ascontiguousarray(
        np.concatenate([bq, bq])[:, None]
    ).astype(f32)                                           # [128, 1]

    shared = {"wkvo": wkvo, "bqd": bqd}
    in_maps = []
    for b in range(B):
        m = dict(shared)
        m["x1"] = np.ascontiguousarray(x1[b].reshape(C1, HW))
        m["x2"] = np.ascontiguousarray(x2[b].reshape(C2, HW))
        in_maps.append(m)
    return in_maps


def run(inputs, trace=False, **trace_kwargs):
    from concourse.bass_utils import run_bass_kernel_spmd

    nc = _get_nc()
    in_maps = _prep_in_maps(**inputs)
    res = run_bass_kernel_spmd(
        nc, in_maps, list(range(B)), trace=trace, **trace_kwargs
    )
    out = np.stack([res.results[i]["out"] for i in range(B)])
    out = out.reshape(B, C1, H, W).astype(np.float32)
    return out, res


def kernel(**inputs) -> np.ndarray:
    out, _ = run(inputs, trace=False)
    return out


# revision 52
# speedup vs baseline: 1.5801x; 1.0029x over previous
"""Trainium2 Bass kernel for pooled cross-attention block (dense_transformer).

Reference computation per batch element b (B=8, one per NeuronCore):
  x2p = 2x2 mean-pool(x2)                      [512, 32, 32]
  Q = Wq @ x1  + bq                            [64, 4096]   (d-part layout)
  K = Wk @ x2p + bk                            [64, 1024]
  V = Wv @ x2p + bv                            [64, 1024]
  attn = softmax_n(Q^T K)                      [4096, 1024]
  out  = attn @ V^T                            [4096, 64]
  y    = out @ Wo^T + bo -> [256, 4096] ; result = x1 + y

Kernel strategy (all on-chip per core, streamed over n in 512-col chunks):
  - scores computed TRANSPOSED: sT[m, n] = K^T Q so softmax's reduce dim m
    is the partition dim; the row-sum r[n] is obtained for free by
    augmenting V^T with a ones column (row 64 of the U = V_aug^T expS
    accumulation).  No PE transposes anywhere.
  - bias algebra (all exact): bk drops (per-row softmax shift invariance);
    bq folded into Q on the PSUM->SBUF copy (DVE tensor_scalar);
    bv folded into bo' = bo + Wo@bv on host (attn rows sum to 1);
    bo' enters via the ones-row of the normalized U against an augmented
    Wo^T.
  - K and Q are produced twice (column-group-tiled matmuls run concurrently
    in the PE array, so the duplicate is ~free) so the scores matmuls can be
    issued as row-group-tiled CONCURRENT pairs: mi even uses array rows
    0-63 / K copy 1 / Q copy 1, mi odd uses rows 64-127 / the duplicates.
    Each pair lands in one 2-bank-wide PSUM tile, consumed by a single
    1024-wide exp ACTIVATE.
  - PE clock: the HAM activity monitor keeps the PE at 1.2 GHz unless it
    sees sustained back-to-back matmul activity.  A dense warmup burst at
    t=0 (during the input DMAs) plus a trickle of dummy matmuls between
    the DMA-paced phase-A bursts keeps the array at 2.4 GHz.
  - 2x2 pooling: two strided DVE adds (bf16); the 1/4 scale is folded into
    Wk/Wv on the host.
  - softmax normalization: 1/r via the fast custom-DVE reciprocal
    (~18 bits), broadcast on GpSimd, applied on DVE; the output projection
    matmuls for chunk j are interleaved into chunk j+1's PE stream so the
    PE never waits on the normalization tail.
"""

import sys

for _p in ("/opt/trn_rl_repo",):
    if _p not in sys.path:
        sys.path.insert(0, _p)

import numpy as np

B, C1, C2, H, W, D = 8, 256, 512, 64, 64, 64
HW = H * W            # n (query) size: 4096
M = (H // 2) * (W // 2)  # kv size: 1024
NCH = 512             # n-chunk (one fp32 PSUM bank)
NCHUNKS = HW // NCH   # 8
C1T = C1 // 128       # 2
C2T = C2 // 128       # 4
MT = M // 128         # 8

DBG = False           # add intermediate-dump outputs (debugging only)

_CACHE = {}


def _build():
    import concourse.bass as bass
    import concourse.tile as tile
    from concourse import bacc, mybir

    dt = mybir.dt
    f32, bf16, f32r = dt.float32, dt.bfloat16, dt.float32r
    Exp = mybir.ActivationFunctionType.Exp

    nc = bacc.Bacc(
        "TRN2", target_bir_lowering=False, debug=False, num_devices=8
    )
    x1 = nc.dram_tensor("x1", [C1, HW], f32, kind="ExternalInput").ap()
    x2 = nc.dram_tensor("x2", [C2, HW], f32, kind="ExternalInput").ap()
    # packed weights: one bf16 blob + one f32 blob -> 2 DMAs total
    wkvo = nc.dram_tensor("wkvo", [128, 896], bf16, kind="ExternalInput").ap()
    bqd = nc.dram_tensor("bqd", [128, 1], f32, kind="ExternalInput").ap()
    out = nc.dram_tensor("out", [C1, HW], f32, kind="ExternalOutput").ap()
    if DBG:
        d_ksb = nc.dram_tensor("d_ksb", [128, M], bf16, kind="ExternalOutput").ap()
        d_qall = nc.dram_tensor(
            "d_qall", [128, NCHUNKS, NCH], bf16, kind="ExternalOutput"
        ).ap()
        d_es = nc.dram_tensor(
            "d_es", [128, 4, 2 * NCH], bf16, kind="ExternalOutput"
        ).ap()
        d_on = nc.dram_tensor("d_on", [D + 1, NCH], bf16, kind="ExternalOutput").ap()
        d_rinv = nc.dram_tensor("d_rinv", [1, NCH], f32, kind="ExternalOutput").ap()
        d_r = nc.dram_tensor("d_r", [1, NCH], f32, kind="ExternalOutput").ap()


    x1v = x1.rearrange("(t p) n -> p t n", p=128)    # [128, 2, HW]
    outv = out.rearrange("(t p) n -> p t n", p=128)  # [128, 2, HW]

    from contextlib import ExitStack

    with tile.TileContext(nc) as tc, ExitStack() as ctx:
        pool = lambda name, bufs, **kw: ctx.enter_context(
            tc.tile_pool(name=name, bufs=bufs, **kw)
        )
        consts = pool("consts", 1)
        warm = pool("warm", 1)
        x2st = pool("x2st", 8)
        x1p = pool("x1p", 8)
        poolp = pool("poolp", 2)
        sbfp = pool("sbfp", 2)
        kvsb = pool("kvsb", 1)
        qallp = pool("qallp", 1)
        esp = pool("esp", 3)
        rp = pool("rp", 2)
        rbp = pool("rbp", 2)
        onp = pool("onp", 2)
        youtp = pool("youtp", 4)
        ps_s = pool("ps_s", 2, space="PSUM")   # [128,1024] wide: 2 banks x 2
        ps_u = pool("ps_u", 2, space="PSUM")   # 1 bank x 2
        ps_qy = pool("ps_qy", 2, space="PSUM")  # 1 bank x 2 (q + y rotate)
        ps_q = ps_y = ps_qy

        # ---- t=0: preload exp table ------------------------------------
        actw = warm.tile([1, 8], f32, tag="actw")
        nc.vector.memset(actw, 0.0)
        actw2 = warm.tile([1, 8], f32, tag="actw2")
        nc.scalar.activation(actw2, actw, Exp)

        # ---- input DMAs: x2 first, 8 halves ordered (hi, ci) -----------
        HHW = HW // 2
        x2t = {}
        for hi in range(2):
            for ci in range(C2T):
                t = x2st.tile([128, HHW], f32, tag="x2t", name=f"x2t{hi}{ci}")
                x2t[hi, ci] = t
                # first transfer via the Scalar HWDGE ring: that queue is
                # free at t=0, so x2 streaming starts ~2us earlier
                eng = nc.scalar if (hi, ci) == (0, 0) else nc.sync
                eng.dma_start(
                    out=t,
                    in_=x2[ci * 128:(ci + 1) * 128, hi * HHW:(hi + 1) * HHW],
                )

        # ---- constants (2 DMAs) ----------------------------------------
        wkvo_sb = consts.tile([128, 896], bf16, tag="wkvo")
        nc.sync.dma_start(out=wkvo_sb, in_=wkvo)
        bq_sb = consts.tile([128, 1], f32, tag="bqd")
        nc.sync.dma_start(out=bq_sb, in_=bqd)
        wkt_sb = wkvo_sb[:, 0:256].rearrange("p (c d) -> p c d", c=C2T)
        wvt_sb = wkvo_sb[:, 256:512].rearrange("p (c d) -> p c d", c=C2T)
        wot_sb = wkvo_sb[:, 512:768]              # [65 used, 256]
        wqt_sb = wkvo_sb[:, 768:896].rearrange("p (t d) -> p t d", t=C1T)

        # ---- x1: SWDGE casting DMAs (fp32->bf16 in the DMA datapath),
        # gated behind the 6th x2 transfer so they don't steal x2's HBM
        # bandwidth on the critical path.  Tile schedules by data deps (not
        # program order), so the gate must be a real WAW dep: a tiny copy
        # of the gate value into each destination tile before its DMA.
        gate_a = warm.tile([1, 1], f32, tag="gate_a")
        nc.gpsimd.tensor_copy(gate_a, x2t[1, 0][0:1, 0:1])
        gate_b = warm.tile([1, 1], f32, tag="gate_b")
        nc.gpsimd.tensor_copy(gate_b, x2t[1, 3][0:1, 0:1])
        x1t = [None] * NCHUNKS
        for nj in range(NCHUNKS):
            x1t[nj] = x1p.tile(
                [128, C1T, NCH], bf16, tag="x1t", name=f"x1t{nj}"
            )
            nc.gpsimd.tensor_copy(
                x1t[nj][0:1, 0:1, 0:1], gate_a if nj < 2 else gate_b
            )
            nc.gpsimd.dma_start(
                out=x1t[nj], in_=x1v[:, :, nj * NCH:(nj + 1) * NCH]
            )

        # ---- phase A: pool x2, project K (col-dup pairs) and V^T -------
        # K's PSUM lives in the q/y pool (1 bank per half) so the s-pool
        # slots are free for the first chunk's score pairs immediately.
        k_pack = [
            ps_qy.tile([128, NCH], f32, tag="qy", name=f"k_pack{h}")
            for h in range(2)
        ]
        v_ps = ps_u.tile([128, MT, D], f32, tag="u", name="v_ps")
        k_sb = kvsb.tile([128, M], bf16, tag="ksb")
        for hi in range(2):
            for ci in range(C2T):
                x2v = x2t[hi, ci].rearrange(
                    "p (h w2 two) -> p h w2 two", w2=W // 2, two=2
                )
                t1 = poolp.tile([128, H // 2, W // 2], bf16, tag="t1", name="t1")
                nc.vector.tensor_add(t1, x2v[:, :, :, 0], x2v[:, :, :, 1])
                t1v = t1.rearrange("p (h2 two) w2 -> p h2 two w2", two=2)
                s_bf = sbfp.tile([128, NCH], bf16, tag="s", name="s_bf")
                s3 = s_bf.rearrange("p (h2 w2) -> p h2 w2", h2=H // 4)
                nc.vector.tensor_add(s3, t1v[:, :, 0, :], t1v[:, :, 1, :])
                first, last = ci == 0, ci == C2T - 1
                # col-group pair: rows 0-63 / 64-127 concurrently
                nc.tensor.matmul(
                    k_pack[hi][0:64, :],
                    lhsT=wkt_sb[:, ci, :], rhs=s_bf, start=first, stop=last,
                    skip_group_check=True,
                )
                nc.tensor.matmul(
                    k_pack[hi][64:128, :],
                    lhsT=wkt_sb[:, ci, :], rhs=s_bf, start=first, stop=last,
                    skip_group_check=True,
                )
                for mj in range(MT // 2):
                    nc.tensor.matmul(
                        v_ps[:, hi * (MT // 2) + mj, :],
                        lhsT=s_bf[:, mj * 128:(mj + 1) * 128],
                        rhs=wvt_sb[:, ci, :],
                        start=first, stop=last,
                        skip_group_check=True,
                    )
            nc.vector.tensor_copy(
                k_sb[:, hi * NCH:(hi + 1) * NCH], k_pack[hi]
            )
        v_aug = kvsb.tile([128, MT, D + 1], bf16, tag="vaug")
        nc.vector.memset(v_aug[:, :, D], 1.0)
        nc.vector.tensor_copy(v_aug[:, :, 0:D], v_ps)

        # ---- Q projection (bf16 col-dup pairs) -------------------------
        q_all = qallp.tile([128, NCHUNKS, NCH], bf16, tag="qall")

        def q_proj(nj):
            q_ps = ps_q.tile([128, NCH], f32, tag="qy", name=f"q_ps{nj}")
            for t in range(C1T):
                nc.tensor.matmul(
                    q_ps[0:64, :], lhsT=wqt_sb[:, t, :], rhs=x1t[nj][:, t, :],
                    start=(t == 0), stop=(t == C1T - 1),
                    skip_group_check=True,
                )
                nc.tensor.matmul(
                    q_ps[64:128, :], lhsT=wqt_sb[:, t, :], rhs=x1t[nj][:, t, :],
                    start=(t == 0), stop=(t == C1T - 1),
                    skip_group_check=True,
                )
            nc.vector.tensor_scalar_add(q_all[:, nj, :], q_ps, bq_sb)

        q_proj(0)
        q_proj(1)

        # ---- phase B: stream n-chunks ----------------------------------
        # PE stream per chunk (software-pipelined so the PE never waits on
        # exp): sp0 sp1 up0 sp2 up1 sp3 [y0'] up2 [y1'] up3, where y' are
        # the previous chunk's output projections.
        pend = None  # (u_ps, nj) awaiting tail + output projection

        def tail_begin(p):
            # normalization tail of the previous chunk (DVE/GpSimd queues)
            u_prev, njp = p
            r_sb = rp.tile([1, NCH], f32, tag="rsb", name="r_sb")
            nc.vector.tensor_copy(r_sb, u_prev[D:D + 1, :])
            rinv = rp.tile([1, NCH], f32, tag="rinv", name="rinv")
            nc.vector.reciprocal_approx_fast(rinv, r_sb)
            if DBG and njp == 0:
                nc.sync.dma_start(out=d_r, in_=r_sb)
                nc.sync.dma_start(out=d_rinv, in_=rinv)
            rb = rbp.tile([D + 1, NCH], f32, tag="rb", name="rb")
            nc.gpsimd.partition_broadcast(rb, rinv)
            on = onp.tile([D + 1, NCH], bf16, tag="on", name="on")
            nc.vector.tensor_mul(on, u_prev, rb)
            if DBG and njp == 0:
                nc.sync.dma_start(out=d_on, in_=on)
            yo = youtp.tile([128, C1T, NCH], f32, tag="yo", name="yo")
            return on, yo, njp

        def tail_y(t, on, yo, njp):
            y_ps = ps_y.tile([128, NCH], f32, tag="qy", name=f"y_ps{njp}_{t}")
            nc.tensor.matmul(
                y_ps, lhsT=wot_sb[0:D + 1, t * 128:(t + 1) * 128], rhs=on,
                start=True, stop=True,
            )
            nc.vector.tensor_add(yo[:, t, :], x1t[njp][:, t, :], y_ps)
            nc.sync.dma_start(
                out=outv[:, t, njp * NCH:(njp + 1) * NCH], in_=yo[:, t, :]
            )

        def tail_flush(on, yo, njp):
            pass

        for nj in range(NCHUNKS):
            if nj + 2 < NCHUNKS:
                q_proj(nj + 2)
            u_ps = ps_u.tile([D + 1, NCH], f32, tag="u", name=f"u_ps{nj}")
            tl = tail_begin(pend) if pend is not None else None

            s_w = [None] * 4
            es = [None] * 4

            def s_pair(p):
                s_w[p] = ps_s.tile([128, 2 * NCH], f32, tag="s", name=f"s_w{nj}_{p}")
                mi_a, mi_b = 2 * p, 2 * p + 1
                nc.tensor.matmul(
                    s_w[p][:, 0:NCH],
                    lhsT=k_sb[0:64, mi_a * 128:(mi_a + 1) * 128],
                    rhs=q_all[0:64, nj, :], start=True, stop=True,
                )
                nc.tensor.matmul(
                    s_w[p][:, NCH:2 * NCH],
                    lhsT=k_sb[64:128, mi_b * 128:(mi_b + 1) * 128],
                    rhs=q_all[64:128, nj, :], start=True, stop=True,
                )
                es[p] = esp.tile([128, 2 * NCH], bf16, tag="es", name=f"es{nj}_{p}")
                nc.scalar.activation(es[p], s_w[p], Exp)
                if DBG and nj == 0:
                    nc.sync.dma_start(out=d_es[:, p, :], in_=es[p])

            def u_pair(p):
                mi_a, mi_b = 2 * p, 2 * p + 1
                nc.tensor.matmul(
                    u_ps, lhsT=v_aug[:, mi_a, :], rhs=es[p][:, 0:NCH],
                    start=(p == 0), stop=False,
                )
                nc.tensor.matmul(
                    u_ps, lhsT=v_aug[:, mi_b, :], rhs=es[p][:, NCH:2 * NCH],
                    start=False, stop=(p == 3),
                )

            s_pair(0)
            s_pair(1)
            u_pair(0)
            s_pair(2)
            u_pair(1)
            s_pair(3)
            if tl is not None:
                tail_y(0, *tl)
            u_pair(2)
            if tl is not None:
                tail_y(1, *tl)
                tail_flush(*tl)
            u_pair(3)
            pend = (u_ps, nj)

        # final chunk: its tail is fully exposed, so pipeline it in two
        # 256-col halves to shorten the serial normalize->project->store
        # chain at the end of the kernel.
        u_prev, njp = pend
        NH = NCH // 2
        r_h, rb_h, on_h = [None] * 2, [None] * 2, [None] * 2
        for hf in range(2):
            sl = slice(hf * NH, (hf + 1) * NH)
            r_sb = rp.tile([1, NH], f32, tag="rsb", name=f"rf{hf}")
            nc.vector.tensor_copy(r_sb, u_prev[D:D + 1, sl])
            rinv = rp.tile([1, NH], f32, tag="rinv", name=f"rif{hf}")
            nc.vector.reciprocal_approx_fast(rinv, r_sb)
            r_h[hf] = rinv
        for hf in range(2):
            rb = rbp.tile([D + 1, NH], f32, tag="rb", name=f"rbf{hf}")
            nc.gpsimd.partition_broadcast(rb, r_h[hf])
            rb_h[hf] = rb
        for hf in range(2):
            sl = slice(hf * NH, (hf + 1) * NH)
            on = onp.tile([D + 1, NH], bf16, tag="on", name=f"onf{hf}")
            nc.vector.tensor_mul(on, u_prev[:, sl], rb_h[hf])
            for t in range(C1T):
                # alternate PSUM slots (s-pool is idle by now) so the four
                # final projections overlap instead of serializing on WAR
                yp_pool = ps_y if (hf + t) % 2 == 0 else ps_s
                yp_tag = "qy" if (hf + t) % 2 == 0 else "s"
                y_ps = yp_pool.tile(
                    [128, NH], f32, tag=yp_tag, name=f"yf{hf}_{t}"
                )
                nc.tensor.matmul(
                    y_ps, lhsT=wot_sb[0:D + 1, t * 128:(t + 1) * 128], rhs=on,
                    start=True, stop=True,
                )
                yo = youtp.tile([128, NH], f32, tag="yo", name=f"yof{hf}_{t}")
                nc.vector.tensor_add(yo, x1t[njp][:, t, sl], y_ps)
                nc.sync.dma_start(
                    out=outv[:, t, njp * NCH + hf * NH:njp * NCH + (hf + 1) * NH],
                    in_=yo,
                )
        if DBG:
            nc.sync.dma_start(out=d_ksb, in_=k_sb)
            nc.sync.dma_start(out=d_qall, in_=q_all)
            nc.sync.dma_start(out=d_x1b, in_=x1b[0])
    nc.compile()
    return nc


def _get_nc():
    if "nc" not in _CACHE:
        _CACHE["nc"] = _build()
    return _CACHE["nc"]


def _prep_in_maps(x1, x2, Wq, bq, Wk, bk, Wv, bv, Wo, bo):
    import ml_dtypes

    bf16 = ml_dtypes.bfloat16
    f32 = np.float32
    x1 = np.asarray(x1, f32)
    x2 = np.asarray(x2, f32)
    Wq = np.asarray(Wq, f32)
    Wk = np.asarray(Wk, f32)
    Wv = np.asarray(Wv, f32)
    Wo = np.asarray(Wo, f32)
    bq = np.asarray(bq, f32)
    bk = np.asarray(bk, f32)
    bv = np.asarray(bv, f32)
    bo = np.asarray(bo, f32)

    # bk is softmax-invariant (constant per score row) and is dropped.
    # bv folds into the output bias because attention rows sum to one.
    bo_eff = bo + Wo @ bv

    def to_p_inner(w):  # [CT*128, D] -> [128, CT*D]
        ct = w.shape[0] // 128
        return np.ascontiguousarray(
            w.reshape(ct, 128, D).transpose(1, 0, 2).reshape(128, ct * D)
        )

    wkt_p = to_p_inner((0.25 * Wk).T)                      # [128, 256]
    wvt_p = to_p_inner((0.25 * Wv).T)                      # [128, 256]
    wot_p = np.zeros((128, 256), f32)
    wot_p[:D + 1] = np.concatenate([Wo.T, bo_eff[None, :]], axis=0)
    wqt_p = to_p_inner(Wq.T)                                # [128, 128]
    wkvo = np.ascontiguousarray(
        np.concatenate([wkt_p, wvt_p, wot_p, wqt_p], axis=1)
    ).astype(bf16)                                          # [128, 896]

    bqd = np.ascontiguousarray(
        np.concatenate([bq, bq])[:, None]
    ).astype(f32)                                           # [128, 1]

    shared = {"wkvo": wkvo, "bqd": bqd}
    in_maps = []
    for b in range(B):
        m = dict(shared)
        m["x1"] = np.ascontiguousarray(x1[b].reshape(C1, HW))
        m["x2"] = np.ascontiguousarray(x2[b].reshape(C2, HW))
        in_maps.append(m)
    return in_maps


def run(inputs, trace=False, **trace_kwargs):
    from concourse.bass_utils import run_bass_kernel_spmd

    nc = _get_nc()
    in_maps = _prep_in_maps(**inputs)
    res = run_bass_kernel_spmd(
        nc, in_maps, list(range(B)), trace=trace, **trace_kwargs
    )
    out = np.stack([res.results[i]["out"] for i in range(B)])
    out = out.reshape(B, C1, H, W).astype(np.float32)
    return out, res


def kernel(**inputs) -> np.ndarray:
    out, _ = run(inputs, trace=False)
    return out
